# revision 10
# baseline (speedup 1.0000x reference)
"""DeepSeek-MLA attention Trainium2 Bass kernel, 8-core SPMD.

Sharding strategy (one NEFF, per-core data differs):
  - Tokens (B*S = 4096) are sharded 512/core for the down-projections and o_proj.
  - Heads (16) are sharded 2/core for the up-projections and attention.
  - Device collectives stitch the two shardings together:
      AllGather(kv_norm^T bf16 + k_rope^T f32r)  after the joint kv down-proj,
      AllGather(q_lora^T bf16) x3                after the q down-proj,
      AllToAll(attn_out^T bf16)                  head-parallel -> token-parallel
  - Projection matmuls run in bf16 (weights cast on host); the attention
    score matmuls (q.k) run in f32r to keep softmax logits accurate.
  - Dataflow is kept feature-major ("T layout": [feature, token]) so no
    transposes are needed anywhere except hidden_states itself (PE transpose).
  - Causal softmax without a running max (scores are O(+-5) after scale, exp
    cannot overflow); masking is an additive -1e9 applied into the score PSUM
    by the vector engine; the denominator l = sum_k P is accumulated on the
    vector engine and partition-reduced on gpsimd; 1/l via the fast DVE
    reciprocal and broadcast across partitions on gpsimd. This keeps the
    tensor engine stream free of bookkeeping matmuls.
  - o_proj is split into a head-0 half (depends only on the first AllToAll)
    and a head-1 half re-seeded from the evicted partials via an identity
    matmul, so the in-order PE queue never stalls on the second AllToAll.

RMSNorm weights are folded into the up-projection weights on the host
(host does only O(d^2) reshuffles + dtype casts; all O(n^3) math on device).
"""

import math

import numpy as np

# ---- problem shapes (hardcoded; harness contract) ----
B, S, HID = 2, 2048, 2048
IN = 2 * HID
H = 16
NOPE, ROPE, VHD = 128, 64, 128
QKD = NOPE + ROPE
QR, KVR = 1536, 512
EPS = 1e-6
THETA = 10000.0
SCALE = 1.0 / math.sqrt(QKD)

NCORES = 8
T = B * S                 # 4096 flat tokens (b-major)
TSH = T // NCORES         # 512 tokens per core
HPC = H // NCORES         # 2 heads per core
NEG = -1.0e9

_cache = {}


def _build():
    import concourse.bass as bass
    import concourse.mybir as mybir
    import concourse.tile as tile
    from concourse import bacc

    dt = mybir.dt
    F32 = dt.float32
    F32R = dt.float32r
    BF16 = dt.bfloat16
    AF = mybir.ActivationFunctionType
    AX = mybir.AxisListType
    ALU = mybir.AluOpType

    nc = bacc.Bacc("TRN2", target_bir_lowering=False, debug=False,
                   num_devices=NCORES)

    # ---------------- I/O ----------------
    def inp(name, shape, dtype=F32R):
        return nc.dram_tensor(name, shape, dtype, kind="ExternalInput").ap()

    hid = inp("hid", [TSH, IN])                       # natural token shard
    wqa = inp("wqa", [IN // 128, 128, QR], BF16)      # full
    wqb = inp("wqb", [QR // 128, 128, HPC * QKD], BF16)   # shard, reordered
    wkva = inp("wkva", [IN // 128, 128, KVR + ROPE], BF16)  # full
    wkvb_kn = inp("wkvb_kn", [KVR // 128, 128, HPC * NOPE], BF16)
    wkvb_v = inp("wkvb_v", [KVR // 128, 128, HPC * VHD], BF16)
    wo = inp("wo", [H * VHD // 128, 128, HID], BF16)  # full
    cos_k = inp("cos_k", [ROPE, TSH], F32)
    sin_k = inp("sin_k", [ROPE, TSH], F32)
    cos_q = inp("cos_q", [2 * ROPE, T], F32)          # doubled for 2 heads
    sin_q = inp("sin_q", [2 * ROPE, T], F32)
    maskb = inp("maskb", [4, 128, 512], F32)          # additive causal biases
    identr = inp("identr", [128, 128])                # f32r identity
    onesc = inp("onesc", [128, 1])
    onescb = inp("onescb", [128, 1], BF16)
    onesr = inp("onesr", [1, 128])
    r128 = inp("r128", [128, 128])                    # q-rope rotation lhsT
    r64 = inp("r64", [ROPE, ROPE])                    # k-rope rotation lhsT

    y = nc.dram_tensor("y", [TSH, HID], F32, kind="ExternalOutput").ap()

    QRC = QR // 128            # 12 q-lora chunks
    KVC = KVR // 128           # 4 kv chunks
    INC = IN // 128            # 32 input chunks
    TC = T // 512              # 8 token chunks (flat)
    SB = S // 512              # 4 token chunks per batch
    NKB = S // 128             # 16 key blocks per batch
    KCO = H * VHD // 128       # 16 o_proj contraction chunks

    with tile.TileContext(nc) as tc:
        with tc.tile_pool(name="dram", bufs=1, space="DRAM") as dram, \
             tc.tile_pool(name="const", bufs=1) as const:

            # ---- DRAM bounce buffers for collectives ----
            ag_kv_in = dram.tile([KVR, TSH], BF16)
            ag_kv_out = dram.tile([NCORES, KVR, TSH], BF16,
                                  addr_space="Shared")
            ag_kr_in = dram.tile([ROPE, TSH], F32R, name="ag_kr_in")
            ag_kr_out = dram.tile([NCORES, ROPE, TSH], F32R,
                                  addr_space="Shared", name="ag_kr_out")
            ag_q_in = [dram.tile([QR // 3, TSH], BF16, name=f"ag_q_in{g}")
                       for g in range(3)]
            ag_q_out = [dram.tile([NCORES, QR // 3, TSH], BF16,
                                  addr_space="Shared", name=f"ag_q_out{g}")
                        for g in range(3)]
            ag_iv_in = dram.tile([1, TSH], F32)
            ag_iv_out = dram.tile([NCORES, 1, TSH], F32, addr_space="Shared")
            a2a_in = [dram.tile([NCORES, VHD, TSH], BF16, name=f"a2a_in{hl}")
                      for hl in range(HPC)]
            a2a_out = [dram.tile([NCORES, VHD, TSH], BF16,
                                 name=f"a2a_out{hl}")
                       for hl in range(HPC)]

            # ---- small constants resident in SBUF ----
            identr_sb = const.tile([128, 128], F32R)
            nc.sync.dma_start(identr_sb[:], identr[:])
            r128_sb = const.tile([128, 128], F32R)
            nc.sync.dma_start(r128_sb[:], r128[:])
            r64_sb = const.tile([ROPE, ROPE], F32R)
            nc.sync.dma_start(r64_sb[:], r64[:])
            onesc_sb = const.tile([128, 1], F32R)
            nc.sync.dma_start(onesc_sb[:], onesc[:])
            onescb_sb = const.tile([128, 1], BF16)
            nc.sync.dma_start(onescb_sb[:], onescb[:])
            onesr_sb = const.tile([1, 128], F32R)
            nc.sync.dma_start(onesr_sb[:], onesr[:])
            cosk_sb = const.tile([ROPE, TSH], F32)
            nc.sync.dma_start(cosk_sb[:], cos_k[:])
            sink_sb = const.tile([ROPE, TSH], F32)
            nc.sync.dma_start(sink_sb[:], sin_k[:])
            maskb_sb = const.tile([128, 4, 512], F32)
            for r in range(4):
                nc.sync.dma_start(maskb_sb[:, r, :], maskb[r])
            eps_sb = const.tile([1, 1], F32)
            nc.vector.memset(eps_sb[:], EPS)

            # ================= phase A/B: hidden^T, down-proj, AGs ========
            with tc.tile_pool(name="ab_sbuf", bufs=1) as ab, \
                 tc.tile_pool(name="ab_w", bufs=6) as abw, \
                 tc.tile_pool(name="ab_stage", bufs=3) as abst:

                # ---- hidden^T via PE transposes (f32r in, bf16 out) ----
                hidT = ab.tile([128, INC, TSH], BF16)   # 32 KB/part
                QIN = IN // 4
                with tc.tile_pool(name="hidn_p", bufs=6) as hidn_p, \
                     tc.tile_pool(name="tp_ps", bufs=3, space="PSUM") as tp_ps:
                    for quarter in range(4):
                        hidn = []
                        for tb in range(TSH // 128):
                            ht = hidn_p.tile([128, QIN], F32R, name="hidn",
                                             tag="hidn")
                            nc.scalar.dma_start(
                                ht[:],
                                hid[tb * 128:(tb + 1) * 128,
                                    quarter * QIN:(quarter + 1) * QIN])
                            hidn.append(ht)
                        for icg in range(QIN // 512):
                            icg_g = quarter * (QIN // 512) + icg
                            for tb in range(TSH // 128):
                                tps = tp_ps.tile([128, 512], F32R, name="tps",
                                                 tag="tps")
                                for j in range(4):
                                    nc.tensor.transpose(
                                        tps[:, j * 128:(j + 1) * 128],
                                        hidn[tb][:, (icg * 4 + j) * 128:
                                                 (icg * 4 + j + 1) * 128],
                                        identr_sb[:])
                                nc.scalar.copy(
                                    hidT[:, icg_g * 4:(icg_g + 1) * 4,
                                         tb * 128:(tb + 1) * 128],
                                    tps.rearrange("p (c t) -> p c t", t=128))

                with tc.tile_pool(name="dp_ps", bufs=5, space="PSUM") as dp_ps, \
                     tc.tile_pool(name="ss_ps", bufs=1, space="PSUM") as ss_ps, \
                     tc.tile_pool(name="ms_ps", bufs=2, space="PSUM") as ms_ps:

                    # ---------- kv down-proj (5 out chunks: 4 kv + rope) ----
                    kv_ps = [dp_ps.tile([128, TSH], F32, name=f"kvps{m}",
                                        tag="dps") for m in range(KVC)]
                    kr_ps = dp_ps.tile([ROPE, TSH], F32, tag="dps")
                    for k in range(INC):
                        wt = abw.tile([128, KVR + ROPE], BF16, name="wkva_t",
                                      tag="wkva_t")
                        nc.scalar.dma_start(wt[:], wkva[k])
                        for m in range(KVC):
                            nc.tensor.matmul(
                                kv_ps[m][:], wt[:, m * 128:(m + 1) * 128],
                                hidT[:, k, :], start=(k == 0), stop=(k == INC - 1))
                        nc.tensor.matmul(
                            kr_ps[:], wt[:, KVR:], hidT[:, k, :],
                            start=(k == 0), stop=(k == INC - 1))

                    # rms over kv chunks
                    kv_raw = [ab.tile([128, TSH], F32, name=f"kvraw{m}",
                                      tag=f"kvraw{m}") for m in range(KVC)]
                    sumsq_kv = ss_ps.tile([1, TSH], F32, tag="ssq")
                    for m in range(KVC):
                        nc.scalar.copy(kv_raw[m][:], kv_ps[m][:])
                        sq = abst.tile([128, TSH], F32R, name="sq", tag="sq")
                        nc.vector.tensor_mul(sq[:], kv_raw[m][:], kv_raw[m][:])
                        nc.tensor.matmul(sumsq_kv[:], onesc_sb[:], sq[:],
                                         start=(m == 0), stop=(m == KVC - 1))
                    s_kv = abst.tile([1, TSH], F32, tag="s_small")
                    nc.scalar.activation(s_kv[:], sumsq_kv[:], AF.Sqrt,
                                         bias=eps_sb[:], scale=1.0 / KVR)
                    inv_kv = abst.tile([1, TSH], F32, tag="inv_small")
                    nc.vector.reciprocal_approx_fast(inv_kv[:], s_kv[:])
                    binv = abst.tile([128, TSH], F32, tag="binv")
                    nc.gpsimd.partition_broadcast(binv[:], inv_kv[:])
                    for m in range(KVC):
                        kvn = abst.tile([128, TSH], BF16, name="kvn", tag="kvn")
                        nc.vector.tensor_mul(kvn[:], kv_raw[m][:], binv[:])
                        nc.sync.dma_start(
                            ag_kv_in[m * 128:(m + 1) * 128, :], kvn[:])

                    # k-rope: rotate + cos/sin (token shard only)
                    krope_raw = ab.tile([ROPE, TSH], F32R)
                    nc.scalar.copy(krope_raw[:], kr_ps[:])
                    rot_ps = ms_ps.tile([ROPE, TSH], F32, tag="msps")
                    nc.tensor.matmul(rot_ps[:], r64_sb[:], krope_raw[:],
                                     start=True, stop=True)
                    t1 = abst.tile([ROPE, TSH], F32, tag="ropet1")
                    nc.vector.tensor_mul(t1[:], krope_raw[:], cosk_sb[:])
                    t2 = abst.tile([ROPE, TSH], F32, tag="ropet2")
                    nc.vector.tensor_mul(t2[:], rot_ps[:], sink_sb[:])
                    krn = abst.tile([ROPE, TSH], F32R, tag="krn")
                    nc.vector.tensor_add(krn[:], t1[:], t2[:])
                    nc.sync.dma_start(ag_kr_in[:], krn[:])

                    nc.gpsimd.collective_compute(
                        "AllGather", mybir.AluOpType.bypass,
                        replica_groups=[list(range(NCORES))],
                        ins=[ag_kv_in.opt()], outs=[ag_kv_out.opt()])
                    nc.gpsimd.collective_compute(
                        "AllGather", mybir.AluOpType.bypass,
                        replica_groups=[list(range(NCORES))],
                        ins=[ag_kr_in.opt()], outs=[ag_kr_out.opt()])

                    # ---------- q down-proj (12 chunks, 3 groups) ----------
                    # RAW (unnormalized) chunks are AllGathered per group as
                    # soon as they finish; the RMS 1/sqrt factor is gathered
                    # separately and folded into the QT up-proj eviction.
                    sumsq_q = ss_ps.tile([1, TSH], F32, tag="ssq")
                    for g in range(3):
                        q_ps = [dp_ps.tile([128, TSH], F32, name=f"qps{m}",
                                           tag="dps") for m in range(4)]
                        for k in range(INC):
                            wt = abw.tile([128, 512], BF16, name="wqa_t",
                                          tag="wqa_t")
                            nc.scalar.dma_start(
                                wt[:], wqa[k, :, g * 512:(g + 1) * 512])
                            for m in range(4):
                                nc.tensor.matmul(
                                    q_ps[m][:], wt[:, m * 128:(m + 1) * 128],
                                    hidT[:, k, :],
                                    start=(k == 0), stop=(k == INC - 1))
                        for m in range(4):
                            mg = g * 4 + m
                            qr_t = abst.tile([128, TSH], BF16, name="qr_t",
                                             tag="qr_t")
                            nc.scalar.copy(qr_t[:], q_ps[m][:])
                            nc.sync.dma_start(
                                ag_q_in[g][m * 128:(m + 1) * 128, :], qr_t[:])
                            sq = abst.tile([128, TSH], F32R, name="sq", tag="sq")
                            nc.vector.tensor_mul(sq[:], qr_t[:], qr_t[:])
                            nc.tensor.matmul(sumsq_q[:], onesc_sb[:], sq[:],
                                             start=(mg == 0),
                                             stop=(mg == QRC - 1))
                        if g == 2:
                            # inv-rms AG goes just before the last (big) q AG
                            s_q = abst.tile([1, TSH], F32, tag="s_small")
                            nc.scalar.activation(s_q[:], sumsq_q[:], AF.Sqrt,
                                                 bias=eps_sb[:], scale=1.0 / QR)
                            inv_q = abst.tile([1, TSH], F32, tag="inv_small")
                            nc.vector.reciprocal_approx_fast(inv_q[:], s_q[:])
                            nc.sync.dma_start(ag_iv_in[:], inv_q[:])
                            nc.gpsimd.collective_compute(
                                "AllGather", mybir.AluOpType.bypass,
                                replica_groups=[list(range(NCORES))],
                                ins=[ag_iv_in.opt()], outs=[ag_iv_out.opt()])
                        nc.gpsimd.collective_compute(
                            "AllGather", mybir.AluOpType.bypass,
                            replica_groups=[list(range(NCORES))],
                            ins=[ag_q_in[g].opt()], outs=[ag_q_out[g].opt()])

            # ============ phase C: up-projections (head-parallel) ==========
            with tc.tile_pool(name="kn_sb", bufs=1) as kn_pool, \
                 tc.tile_pool(name="v_sb", bufs=1) as v_pool, \
                 tc.tile_pool(name="qt_sb", bufs=1) as qt_pool, \
                 tc.tile_pool(name="kr_sb", bufs=1) as kr_pool:

                knT = kn_pool.tile([128, HPC, TC, 512], F32R)   # 32 KB/part
                v_sb = v_pool.tile([128, TSH // 128 * NCORES, HPC * VHD], BF16)
                qT = qt_pool.tile([128, 3, TC, 512], F32R)      # 48 KB/part
                # k_rope^T doubled into both partition halves so that the
                # rope score matmul's lhsT base_partition matches q's half
                krT = kr_pool.tile([2 * ROPE, TC, 512], F32R)
                nc.sync.dma_start(
                    krT[0:ROPE, :, :],
                    ag_kr_out.transpose([1, 0, 2]))
                nc.sync.dma_start(
                    krT[ROPE:, :, :],
                    ag_kr_out.transpose([1, 0, 2]))

                with tc.tile_pool(name="kvn_sb", bufs=8) as kvn_pool, \
                     tc.tile_pool(name="upw", bufs=1) as upw, \
                     tc.tile_pool(name="up_ps", bufs=6, space="PSUM") as up_ps:
                    wkn_sb = upw.tile([128, KVC, HPC * NOPE], BF16)
                    for k in range(KVC):
                        nc.scalar.dma_start(wkn_sb[:, k, :], wkvb_kn[k])
                    wv_sb = upw.tile([128, KVC, HPC * VHD], BF16)
                    for k in range(KVC):
                        nc.scalar.dma_start(wv_sb[:, k, :], wkvb_v[k])

                    # K_nope^T and V, streaming kv_norm^T tiles from the AG
                    for tcb in range(TC):
                        rh = []
                        for k in range(KVC):
                            rt = kvn_pool.tile([128, 512], BF16, name="kvn_t",
                                               tag="kvn_t")
                            nc.sync.dma_start(
                                rt[:],
                                ag_kv_out[tcb, k * 128:(k + 1) * 128, :])
                            rh.append(rt)
                        psn = [up_ps.tile([128, 512], F32, name=f"knps{hl}",
                                          tag="upps") for hl in range(HPC)]
                        for k in range(KVC):
                            for hl in range(HPC):
                                nc.tensor.matmul(
                                    psn[hl][:],
                                    wkn_sb[:, k, hl * NOPE:(hl + 1) * NOPE],
                                    rh[k][:],
                                    start=(k == 0), stop=(k == KVC - 1))
                        for hl in range(HPC):
                            nc.scalar.copy(knT[:, hl, tcb, :], psn[hl][:])
                        psv = [up_ps.tile([128, HPC * VHD], F32,
                                          name=f"vps{j}", tag="upps")
                               for j in range(4)]
                        for k in range(KVC):
                            for j in range(4):
                                nc.tensor.matmul(
                                    psv[j][:],
                                    rh[k][:, j * 128:(j + 1) * 128],
                                    wv_sb[:, k, :],
                                    start=(k == 0), stop=(k == KVC - 1))
                        for j in range(4):
                            nc.vector.tensor_copy(v_sb[:, tcb * 4 + j, :],
                                                  psv[j][:])

                # Q^T (3 chunks: nope h0, nope h1, rope doubled), with the
                # deferred RMS normalize folded into the PSUM eviction and
                # rope applied per token-chunk right after.
                with tc.tile_pool(name="agq_sb", bufs=6) as agq_pool, \
                     tc.tile_pool(name="qw_sb", bufs=1) as qw_pool, \
                     tc.tile_pool(name="rope_sb", bufs=2) as rope_pool, \
                     tc.tile_pool(name="ropest", bufs=2) as ropest, \
                     tc.tile_pool(name="qt_ps", bufs=4, space="PSUM") as qt_ps, \
                     tc.tile_pool(name="rr_ps", bufs=2, space="PSUM") as rr_ps:
                    wqb_sb = qw_pool.tile([128, QRC, HPC * QKD], BF16)
                    for k in range(QRC):
                        nc.scalar.dma_start(wqb_sb[:, k, :], wqb[k])
                    invq_sb = qw_pool.tile([1, TC, 512], F32)
                    nc.sync.dma_start(
                        invq_sb[:],
                        ag_iv_out.rearrange("r o t -> o r t"))
                    for tcb in range(TC):
                        ps = [qt_ps.tile([128, 512], F32, name=f"qtps{m}",
                                         tag="qtps") for m in range(3)]
                        for k in range(QRC):
                            rh16 = agq_pool.tile([128, 512], BF16, name="agq16",
                                                 tag="agq16")
                            nc.sync.dma_start(
                                rh16[:],
                                ag_q_out[k // 4][tcb,
                                                 (k % 4) * 128:(k % 4 + 1) * 128,
                                                 :])
                            for m in range(3):
                                nc.tensor.matmul(
                                    ps[m][:],
                                    wqb_sb[:, k, m * 128:(m + 1) * 128],
                                    rh16[:],
                                    start=(k == 0), stop=(k == QRC - 1))
                        # broadcast 1/rms across partitions on gpsimd, then
                        # evict with the normalize multiply
                        biq_sb = ropest.tile([128, 512], F32, name="biq_sb",
                                             tag="biq_sb")
                        nc.gpsimd.partition_broadcast(biq_sb[:],
                                                      invq_sb[:, tcb, :])
                        for m in range(3):
                            nc.vector.tensor_mul(qT[:, m, tcb, :], ps[m][:],
                                                 biq_sb[:])
                        # q-rope on chunk m=2 (both heads doubled)
                        cosq_t = rope_pool.tile([128, 512], F32, name="cosq_t",
                                                tag="cosq_t")
                        nc.sync.dma_start(cosq_t[:],
                                          cos_q[:, tcb * 512:(tcb + 1) * 512])
                        sinq_t = rope_pool.tile([128, 512], F32, name="sinq_t",
                                                tag="sinq_t")
                        nc.sync.dma_start(sinq_t[:],
                                          sin_q[:, tcb * 512:(tcb + 1) * 512])
                        rps = rr_ps.tile([128, 512], F32, tag="rrps")
                        nc.tensor.matmul(rps[:], r128_sb[:], qT[:, 2, tcb, :],
                                         start=True, stop=True)
                        t1 = ropest.tile([128, 512], F32, name="rt1", tag="rt1")
                        nc.vector.tensor_mul(t1[:], qT[:, 2, tcb, :], cosq_t[:])
                        t2 = ropest.tile([128, 512], F32, name="rt2", tag="rt2")
                        nc.vector.tensor_mul(t2[:], rps[:], sinq_t[:])
                        nc.vector.tensor_add(qT[:, 2, tcb, :], t1[:], t2[:])

                # ============ phase D: causal attention =================
                with tc.tile_pool(name="ao_sb", bufs=1) as ao_pool, \
                     tc.tile_pool(name="wo_sb", bufs=4) as wo_pool:
                  aosb = ao_pool.tile([128, KCO, 512], BF16)
                  with tc.tile_pool(name="pt_sb", bufs=6) as pt_pool, \
                     tc.tile_pool(name="att_st", bufs=2) as att_st, \
                     tc.tile_pool(name="st_ps", bufs=4, space="PSUM") as st_ps, \
                     tc.tile_pool(name="l_ps", bufs=2, space="PSUM") as l_ps, \
                     tc.tile_pool(name="o_ps", bufs=2, space="PSUM") as o_ps:
                    for hl in range(HPC):
                        for b in range(B):
                            for qc in range(SB):
                                tcq = b * SB + qc
                                nkb = 4 * qc + 4
                                op = o_ps.tile([128, 512], F32, name="op",
                                               tag="op")
                                lp = l_ps.tile([1, 512], F32, name="lp",
                                               tag="lp")

                                # software-pipelined: lp/op of block kb are
                                # emitted after the score matmuls of kb+1 so
                                # the PE never waits on the scalar-engine exp
                                def lp_op(kb, pt):
                                    nc.tensor.matmul(lp[:], onescb_sb[:],
                                                     pt[:],
                                                     start=(kb == 0),
                                                     stop=(kb == nkb - 1))
                                    nc.tensor.matmul(
                                        op[:],
                                        v_sb[:, b * NKB + kb,
                                             hl * VHD:(hl + 1) * VHD],
                                        pt[:],
                                        start=(kb == 0), stop=(kb == nkb - 1))

                                prev = None
                                for kb in range(nkb):
                                    tck = b * SB + kb // 4
                                    co = (kb % 4) * 128
                                    st = st_ps.tile([128, 512], F32,
                                                    name="st", tag="st")
                                    diag = kb >= 4 * qc
                                    nc.tensor.matmul(
                                        st[:],
                                        knT[:, hl, tck, co:co + 128],
                                        qT[:, hl, tcq, :],
                                        start=True, stop=False)
                                    nc.tensor.matmul(
                                        st[:],
                                        krT[hl * ROPE:(hl + 1) * ROPE,
                                            tck, co:co + 128],
                                        qT[hl * ROPE:(hl + 1) * ROPE, 2, tcq, :],
                                        start=False, stop=True)
                                    if prev is not None:
                                        lp_op(*prev)
                                    if diag:
                                        nc.vector.tensor_add(
                                            st[:], st[:],
                                            maskb_sb[:, kb - 4 * qc, :])
                                    pt = pt_pool.tile([128, 512], BF16,
                                                      name="pt", tag="pt")
                                    nc.scalar.activation(pt[:], st[:], AF.Exp,
                                                         scale=SCALE)
                                    prev = (kb, pt)
                                lp_op(*prev)
                                invl = att_st.tile([1, 512], F32, name="invl",
                                                   tag="invl")
                                nc.vector.reciprocal_approx_fast(invl[:], lp[:])
                                bi = att_st.tile([128, 512], F32, name="bi",
                                                 tag="bi")
                                nc.gpsimd.partition_broadcast(bi[:], invl[:])
                                att = att_st.tile([128, 512], BF16, name="att",
                                                  tag="att")
                                nc.vector.tensor_mul(att[:], op[:], bi[:])
                                nc.sync.dma_start(
                                    a2a_in[hl][tcq, :, :], att[:])
                        nc.gpsimd.collective_compute(
                            "AllToAll", mybir.AluOpType.bypass,
                            replica_groups=[list(range(NCORES))],
                            ins=[a2a_in[hl].opt()], outs=[a2a_out[hl].opt()])
                        for i in range(NCORES):
                            nc.sync.dma_start(aosb[:, 2 * i + hl, :],
                                              a2a_out[hl][i])

                  # ============ phase E: o_proj (token-parallel) ===========
                  # Two passes: head-0 contraction chunks (depend only on the
                  # first AllToAll) accumulate + evict partials; the head-1
                  # pass re-seeds PSUM from the partials via identity matmul.
                  with tc.tile_pool(name="yp_sb", bufs=1) as yp_pool, \
                       tc.tile_pool(name="yo_sb", bufs=3) as yo_pool, \
                       tc.tile_pool(name="op_ps", bufs=8, space="PSUM") as op_ps:
                    yp = yp_pool.tile([128, HID // 512, 4, 512], F32R)
                    kc_h0 = [2 * i for i in range(NCORES)]
                    kc_h1 = [2 * i + 1 for i in range(NCORES)]
                    for n in range(HID // 512):
                        pso = [op_ps.tile([128, 512], F32, name=f"pso{mt}",
                                          tag="pso") for mt in range(4)]
                        for ki, kc in enumerate(kc_h0):
                            wot = wo_pool.tile([128, 512], BF16, name="wot",
                                               tag="wot")
                            nc.scalar.dma_start(
                                wot[:], wo[kc, :, n * 512:(n + 1) * 512])
                            for mt in range(4):
                                nc.tensor.matmul(
                                    pso[mt][:],
                                    aosb[:, kc, mt * 128:(mt + 1) * 128],
                                    wot[:],
                                    start=(ki == 0), stop=(ki == NCORES - 1))
                        for mt in range(4):
                            nc.scalar.copy(yp[:, n, mt, :], pso[mt][:])
                    for n in range(HID // 512):
                        pso = [op_ps.tile([128, 512], F32, name=f"ps2{mt}",
                                          tag="pso") for mt in range(4)]
                        for mt in range(4):
                            nc.tensor.matmul(pso[mt][:], identr_sb[:],
                                             yp[:, n, mt, :],
                                             start=True, stop=False)
                        for ki, kc in enumerate(kc_h1):
                            wot = wo_pool.tile([128, 512], BF16, name="wot",
                                               tag="wot")
                            nc.scalar.dma_start(
                                wot[:], wo[kc, :, n * 512:(n + 1) * 512])
                            for mt in range(4):
                                nc.tensor.matmul(
                                    pso[mt][:],
                                    aosb[:, kc, mt * 128:(mt + 1) * 128],
                                    wot[:],
                                    start=False, stop=(ki == NCORES - 1))
                        for mt in range(4):
                            yt = yo_pool.tile([128, 512], F32, name="yt",
                                              tag="yt")
                            nc.scalar.copy(yt[:], pso[mt][:])
                            nc.sync.dma_start(
                                y[mt * 128:(mt + 1) * 128,
                                  n * 512:(n + 1) * 512], yt[:])

    nc.compile()
    return nc


def _rot_lhsT(n):
    """lhsT for the interleaved rotate-half as a matmul: out = R @ x,
    R[2i, 2i+1] = -1, R[2i+1, 2i] = +1; matmul computes lhsT.T @ rhs."""
    R = np.zeros((n, n), dtype=np.float32)
    for i in range(n // 2):
        R[2 * i, 2 * i + 1] = -1.0
        R[2 * i + 1, 2 * i] = 1.0
    return np.ascontiguousarray(R.T)


def _prep_inputs(inputs):
    """Host-side sharding/reordering. Returns in_maps (list of 8 dicts)."""
    import ml_dtypes
    bf16 = ml_dtypes.bfloat16

    hs = np.ascontiguousarray(
        np.asarray(inputs["hidden_states"], dtype=np.float32).reshape(T, IN))
    Wq_a = np.asarray(inputs["Wq_a"], dtype=np.float32)
    q_a_ln = np.asarray(inputs["q_a_ln"], dtype=np.float32)
    Wq_b = np.asarray(inputs["Wq_b"], dtype=np.float32) * q_a_ln[:, None]
    Wkv_a = np.asarray(inputs["Wkv_a"], dtype=np.float32)
    kv_a_ln = np.asarray(inputs["kv_a_ln"], dtype=np.float32)
    Wkv_b = np.asarray(inputs["Wkv_b"], dtype=np.float32) * kv_a_ln[:, None]
    Wo = np.asarray(inputs["Wo"], dtype=np.float32)
    pos = np.asarray(inputs["position_ids"]).astype(np.float64)   # [B, S]

    # rope tables (doubled pairs): cos[2i] = cos[2i+1] = cos(pos * invf_i)
    invf = 1.0 / (THETA ** (np.arange(0, ROPE, 2, dtype=np.float64) / ROPE))
    fr = pos[..., None] * invf                       # [B, S, 32]
    cosd = np.repeat(np.cos(fr), 2, axis=-1).astype(np.float32)  # [B, S, 64]
    sind = np.repeat(np.sin(fr), 2, axis=-1).astype(np.float32)
    cosT = np.ascontiguousarray(cosd.reshape(T, ROPE).T)   # [64, T]
    sinT = np.ascontiguousarray(sind.reshape(T, ROPE).T)
    cos_q = np.concatenate([cosT, cosT], axis=0)           # [128, T]
    sin_q = np.concatenate([sinT, sinT], axis=0)

    # causal additive mask biases for diagonal blocks
    maskb = np.zeros((4, 128, 512), dtype=np.float32)
    kl = np.arange(128)[:, None]
    ql = np.arange(512)[None, :]
    for r in range(4):
        maskb[r] = np.where(kl + 128 * r > ql, NEG, 0.0)

    ident = np.eye(128, dtype=np.float32)
    onesc = np.ones((128, 1), dtype=np.float32)
    onesr = np.ones((1, 128), dtype=np.float32)

    wqa_r = np.ascontiguousarray(
        Wq_a.reshape(IN // 128, 128, QR).astype(bf16))
    wkva_r = np.ascontiguousarray(
        Wkv_a.reshape(IN // 128, 128, KVR + ROPE).astype(bf16))
    wo_r = np.ascontiguousarray(
        Wo.reshape(H * VHD // 128, 128, HID).astype(bf16))

    Wq_b_h = Wq_b.reshape(QR, H, QKD)
    Wkv_b_h = Wkv_b.reshape(KVR, H, NOPE + VHD)

    in_maps = []
    for c in range(NCORES):
        h0, h1 = HPC * c, HPC * c + 1
        bc = c // (NCORES // B)
        s0 = (c % (NCORES // B)) * TSH
        # reorder q_b cols: [nope_h0 | nope_h1 | rope_h0 ; rope_h1]
        wqb_s = np.concatenate([
            Wq_b_h[:, h0, :NOPE], Wq_b_h[:, h1, :NOPE],
            Wq_b_h[:, h0, NOPE:], Wq_b_h[:, h1, NOPE:]], axis=1)
        wqb_s = np.ascontiguousarray(
            wqb_s.reshape(QR // 128, 128, HPC * QKD).astype(bf16))
        wkvb_kn_s = np.ascontiguousarray(
            np.concatenate([Wkv_b_h[:, h0, :NOPE], Wkv_b_h[:, h1, :NOPE]],
                           axis=1).reshape(KVR // 128, 128, HPC * NOPE)
            .astype(bf16))
        wkvb_v_s = np.ascontiguousarray(
            np.concatenate([Wkv_b_h[:, h0, NOPE:], Wkv_b_h[:, h1, NOPE:]],
                           axis=1).reshape(KVR // 128, 128, HPC * VHD)
            .astype(bf16))
        tok0 = c * TSH
        in_maps.append({
            "hid": np.ascontiguousarray(hs[tok0:tok0 + TSH]),
            "wqa": wqa_r, "wqb": wqb_s, "wkva": wkva_r,
            "wkvb_kn": wkvb_kn_s, "wkvb_v": wkvb_v_s, "wo": wo_r,
            "cos_k": np.ascontiguousarray(
                cosT[:, bc * S + s0: bc * S + s0 + TSH]),
            "sin_k": np.ascontiguousarray(
                sinT[:, bc * S + s0: bc * S + s0 + TSH]),
            "cos_q": cos_q, "sin_q": sin_q,
            "maskb": maskb, "identr": ident,
            "r128": np.ascontiguousarray(
                np.block([[_rot_lhsT(ROPE), np.zeros((ROPE, ROPE), np.float32)],
                          [np.zeros((ROPE, ROPE), np.float32), _rot_lhsT(ROPE)]])),
            "r64": _rot_lhsT(ROPE),
            "onesc": onesc, "onescb": onesc.astype(bf16), "onesr": onesr,
        })
    return in_maps


def kernel(**inputs) -> np.ndarray:
    from concourse.bass_utils import run_bass_kernel_spmd

    if "nc" not in _cache:
        _cache["nc"] = _build()
    nc = _cache["nc"]
    in_maps = _prep_inputs(inputs)
    res = run_bass_kernel_spmd(nc, in_maps, core_ids=list(range(NCORES)))
    out = np.concatenate([res.results[c]["y"] for c in range(NCORES)], axis=0)
    return np.ascontiguousarray(out.reshape(B, S, HID))


if __name__ == "__main__":
    rng = np.random.default_rng(0)
    ins = {
        "hidden_states": rng.standard_normal((B, S, IN), dtype=np.float32),
        "Wq_a": rng.standard_normal((IN, QR), dtype=np.float32) * IN ** -0.5,
        "q_a_ln": np.ones(QR, np.float32),
        "Wq_b": rng.standard_normal((QR, H * QKD), dtype=np.float32) * QR ** -0.5,
        "Wkv_a": rng.standard_normal((IN, KVR + ROPE), dtype=np.float32) * IN ** -0.5,
        "kv_a_ln": np.ones(KVR, np.float32),
        "Wkv_b": rng.standard_normal((KVR, H * (NOPE + VHD)), dtype=np.float32) * KVR ** -0.5,
        "Wo": rng.standard_normal((H * VHD, HID), dtype=np.float32) * (H * VHD) ** -0.5,
        "position_ids": np.tile(np.arange(S, dtype=np.int32)[None], (B, 1)),
    }
    out = kernel(**ins)
    print("kernel ran, out shape", out.shape, "absmax", np.abs(out).max())


# revision 25
# speedup vs baseline: 1.0269x; 1.0269x over previous
"""DeepSeek-MLA attention Trainium2 Bass kernel, 8-core SPMD.

Sharding strategy (one NEFF, per-core data differs):
  - Tokens (B*S = 4096) are sharded 512/core for the down-projections and o_proj.
  - Heads (16) are sharded 2/core for the up-projections and attention.
  - Device collectives stitch the two shardings together:
      AllGather(kv_norm^T bf16 + k_rope^T f32r)  after the joint kv down-proj,
      AllGather(q_lora^T bf16) x3                after the q down-proj,
      AllToAll(attn_out^T bf16)                  head-parallel -> token-parallel
  - Projection matmuls run in bf16 (weights cast on host); the attention
    score matmuls (q.k) run in f32r to keep softmax logits accurate.
  - Dataflow is kept feature-major ("T layout": [feature, token]) so no
    transposes are needed anywhere except hidden_states itself (PE transpose).
  - Causal softmax without a running max (scores are O(+-5) after scale, exp
    cannot overflow); masking is an additive -1e9 applied into the score PSUM
    by the vector engine; the denominator l = sum_k P is accumulated on the
    vector engine and partition-reduced on gpsimd; 1/l via the fast DVE
    reciprocal and broadcast across partitions on gpsimd. This keeps the
    tensor engine stream free of bookkeeping matmuls.
  - o_proj is split into a head-0 half (depends only on the first AllToAll)
    and a head-1 half re-seeded from the evicted partials via an identity
    matmul, so the in-order PE queue never stalls on the second AllToAll.

RMSNorm weights are folded into the up-projection weights on the host
(host does only O(d^2) reshuffles + dtype casts; all O(n^3) math on device).
"""

import math

import numpy as np

# ---- problem shapes (hardcoded; harness contract) ----
B, S, HID = 2, 2048, 2048
IN = 2 * HID
H = 16
NOPE, ROPE, VHD = 128, 64, 128
QKD = NOPE + ROPE
QR, KVR = 1536, 512
EPS = 1e-6
THETA = 10000.0
SCALE = 1.0 / math.sqrt(QKD)

NCORES = 8
T = B * S                 # 4096 flat tokens (b-major)
TSH = T // NCORES         # 512 tokens per core
HPC = H // NCORES         # 2 heads per core
NEG = -1.0e9

_cache = {}


def _build():
    import concourse.bass as bass
    import concourse.mybir as mybir
    import concourse.tile as tile
    from concourse import bacc

    dt = mybir.dt
    F32 = dt.float32
    F32R = dt.float32r
    BF16 = dt.bfloat16
    AF = mybir.ActivationFunctionType
    AX = mybir.AxisListType
    ALU = mybir.AluOpType

    nc = bacc.Bacc("TRN2", target_bir_lowering=False, debug=False,
                   num_devices=NCORES)

    # ---------------- I/O ----------------
    def inp(name, shape, dtype=F32R):
        return nc.dram_tensor(name, shape, dtype, kind="ExternalInput").ap()

    hid = inp("hid", [TSH, IN])                       # natural token shard
    wqa = inp("wqa", [IN // 128, 128, QR], BF16)      # full
    wqb = inp("wqb", [QR // 128, 128, HPC * QKD], BF16)   # shard, reordered
    wkva = inp("wkva", [IN // 128, 128, KVR + ROPE], BF16)  # full
    wkvb_kn = inp("wkvb_kn", [KVR // 128, 128, HPC * NOPE], BF16)
    wkvb_v = inp("wkvb_v", [KVR // 128, 128, HPC * VHD], BF16)
    wo = inp("wo", [H * VHD // 128, 128, HID], BF16)  # full
    cos_k = inp("cos_k", [ROPE, TSH], F32)
    sin_k = inp("sin_k", [ROPE, TSH], F32)
    cos_q = inp("cos_q", [2 * ROPE, T], F32)          # doubled for 2 heads
    sin_q = inp("sin_q", [2 * ROPE, T], F32)
    maskb = inp("maskb", [4, 128, 512], F32)          # additive causal biases
    identr = inp("identr", [128, 128])                # f32r identity
    onesc = inp("onesc", [128, 1])
    onescb = inp("onescb", [128, 1], BF16)
    onesr = inp("onesr", [1, 128])
    r128 = inp("r128", [128, 128])                    # q-rope rotation lhsT
    r64 = inp("r64", [ROPE, ROPE])                    # k-rope rotation lhsT

    y = nc.dram_tensor("y", [TSH, HID], F32, kind="ExternalOutput").ap()

    QRC = QR // 128            # 12 q-lora chunks
    KVC = KVR // 128           # 4 kv chunks
    INC = IN // 128            # 32 input chunks
    TC = T // 512              # 8 token chunks (flat)
    SB = S // 512              # 4 token chunks per batch
    NKB = S // 128             # 16 key blocks per batch
    KCO = H * VHD // 128       # 16 o_proj contraction chunks

    with tile.TileContext(nc) as tc:
        with tc.tile_pool(name="dram", bufs=1, space="DRAM") as dram, \
             tc.tile_pool(name="const", bufs=1) as const:

            # ---- DRAM bounce buffers for collectives ----
            ag_kv_in = dram.tile([KVR, TSH], BF16)
            ag_kv_out = dram.tile([NCORES, KVR, TSH], BF16,
                                  addr_space="Shared")
            ag_kr_in = dram.tile([ROPE, TSH], BF16, name="ag_kr_in")
            ag_kr_out = dram.tile([NCORES, ROPE, TSH], BF16,
                                  addr_space="Shared", name="ag_kr_out")
            NQG = 6                      # q AllGather groups (2 chunks each)
            ag_q_in = [dram.tile([QR // NQG, TSH], BF16, name=f"ag_q_in{g}")
                       for g in range(NQG)]
            ag_q_out = [dram.tile([NCORES, QR // NQG, TSH], BF16,
                                  addr_space="Shared", name=f"ag_q_out{g}")
                        for g in range(NQG)]
            ag_iv_in = dram.tile([1, TSH], F32)
            ag_iv_out = dram.tile([NCORES, 1, TSH], F32, addr_space="Shared")
            a2a_in = [dram.tile([NCORES, VHD, TSH], BF16, name=f"a2a_in{hl}")
                      for hl in range(HPC)]
            a2a_out = [dram.tile([NCORES, VHD, TSH], BF16,
                                 name=f"a2a_out{hl}")
                       for hl in range(HPC)]

            # ---- small constants resident in SBUF ----
            identr_sb = const.tile([128, 128], F32R)
            nc.sync.dma_start(identr_sb[:], identr[:])
            r128_sb = const.tile([128, 128], F32R)
            nc.sync.dma_start(r128_sb[:], r128[:])
            r64_sb = const.tile([ROPE, ROPE], F32R)
            nc.sync.dma_start(r64_sb[:], r64[:])
            onesc_sb = const.tile([128, 1], F32R)
            nc.sync.dma_start(onesc_sb[:], onesc[:])
            onescb_sb = const.tile([128, 1], BF16)
            nc.sync.dma_start(onescb_sb[:], onescb[:])
            onesr_sb = const.tile([1, 128], F32R)
            nc.sync.dma_start(onesr_sb[:], onesr[:])
            cosk_sb = const.tile([ROPE, TSH], F32)
            nc.sync.dma_start(cosk_sb[:], cos_k[:])
            sink_sb = const.tile([ROPE, TSH], F32)
            nc.sync.dma_start(sink_sb[:], sin_k[:])
            maskb_sb = const.tile([128, 4, 512], F32)
            for r in range(4):
                nc.sync.dma_start(maskb_sb[:, r, :], maskb[r])
            eps_sb = const.tile([1, 1], F32)
            nc.vector.memset(eps_sb[:], EPS)

            # ================= phase A/B: hidden^T, down-proj, AGs ========
            with tc.tile_pool(name="ab_sbuf", bufs=1) as ab, \
                 tc.tile_pool(name="ab_w", bufs=12) as abw, \
                 tc.tile_pool(name="ab_stage", bufs=3) as abst:

                # ---- hidden^T via PE transposes (f32r in, bf16 out) ----
                # full-row hid tiles: 16KB-contiguous DMA descriptors, spread
                # over two IO queues so the transposes are never DMA-starved
                hidT = ab.tile([128, INC, TSH], BF16)   # 32 KB/part
                with tc.tile_pool(name="hidn_p", bufs=4) as hidn_p, \
                     tc.tile_pool(name="tp_ps", bufs=3, space="PSUM") as tp_ps:
                    hidn = []
                    for tb in range(TSH // 128):
                        ht = hidn_p.tile([128, IN], F32R, name="hidn",
                                         tag="hidn")
                        eng = nc.sync if tb % 2 else nc.scalar
                        eng.dma_start(ht[:], hid[tb * 128:(tb + 1) * 128, :])
                        hidn.append(ht)
                    for icg_g in range(INC // 4):
                        for tb in range(TSH // 128):
                            tps = tp_ps.tile([128, 512], F32R, name="tps",
                                             tag="tps")
                            for j in range(4):
                                nc.tensor.transpose(
                                    tps[:, j * 128:(j + 1) * 128],
                                    hidn[tb][:, (icg_g * 4 + j) * 128:
                                             (icg_g * 4 + j + 1) * 128],
                                    identr_sb[:])
                            nc.scalar.copy(
                                hidT[:, icg_g * 4:(icg_g + 1) * 4,
                                     tb * 128:(tb + 1) * 128],
                                tps.rearrange("p (c t) -> p c t", t=128))

                with tc.tile_pool(name="dp_ps", bufs=6, space="PSUM") as dp_ps, \
                     tc.tile_pool(name="ss_ps", bufs=1, space="PSUM") as ss_ps, \
                     tc.tile_pool(name="ms_ps", bufs=1, space="PSUM") as ms_ps:

                    # ---------- kv down-proj (5 out chunks: 4 kv + rope) ----
                    kv_ps = [dp_ps.tile([128, TSH], F32, name=f"kvps{m}",
                                        tag="dps") for m in range(KVC)]
                    kr_ps = dp_ps.tile([ROPE, TSH], F32, tag="dps")
                    for k in range(INC):
                        wt = abw.tile([128, KVR + ROPE], BF16, name="wkva_t",
                                      tag="wkva_t")
                        nc.scalar.dma_start(wt[:], wkva[k])
                        for m in range(KVC):
                            nc.tensor.matmul(
                                kv_ps[m][:], wt[:, m * 128:(m + 1) * 128],
                                hidT[:, k, :], start=(k == 0), stop=(k == INC - 1))
                        nc.tensor.matmul(
                            kr_ps[:], wt[:, KVR:], hidT[:, k, :],
                            start=(k == 0), stop=(k == INC - 1))

                    # rms over kv chunks
                    kv_raw = [ab.tile([128, TSH], F32, name=f"kvraw{m}",
                                      tag=f"kvraw{m}") for m in range(KVC)]
                    sumsq_kv = ss_ps.tile([1, TSH], F32, tag="ssq")
                    for m in range(KVC):
                        nc.scalar.copy(kv_raw[m][:], kv_ps[m][:])
                        sq = abst.tile([128, TSH], F32R, name="sq", tag="sq")
                        nc.vector.tensor_mul(sq[:], kv_raw[m][:], kv_raw[m][:])
                        nc.tensor.matmul(sumsq_kv[:], onesc_sb[:], sq[:],
                                         start=(m == 0), stop=(m == KVC - 1))
                    s_kv = abst.tile([1, TSH], F32, tag="s_small")
                    nc.scalar.activation(s_kv[:], sumsq_kv[:], AF.Sqrt,
                                         bias=eps_sb[:], scale=1.0 / KVR)
                    inv_kv = abst.tile([1, TSH], F32, tag="inv_small")
                    nc.vector.reciprocal_approx_fast(inv_kv[:], s_kv[:])
                    binv = abst.tile([128, TSH], F32, tag="binv")
                    nc.gpsimd.partition_broadcast(binv[:], inv_kv[:])
                    for m in range(KVC):
                        kvn = abst.tile([128, TSH], BF16, name="kvn", tag="kvn")
                        nc.vector.tensor_mul(kvn[:], kv_raw[m][:], binv[:])
                        nc.sync.dma_start(
                            ag_kv_in[m * 128:(m + 1) * 128, :], kvn[:])

                    # k-rope: rotate + cos/sin (token shard only)
                    krope_raw = ab.tile([ROPE, TSH], F32R)
                    nc.scalar.copy(krope_raw[:], kr_ps[:])
                    rot_ps = ms_ps.tile([ROPE, TSH], F32, tag="msps")
                    nc.tensor.matmul(rot_ps[:], r64_sb[:], krope_raw[:],
                                     start=True, stop=True)
                    t1 = abst.tile([ROPE, TSH], F32, tag="ropet1")
                    nc.vector.tensor_mul(t1[:], krope_raw[:], cosk_sb[:])
                    t2 = abst.tile([ROPE, TSH], F32, tag="ropet2")
                    nc.vector.tensor_mul(t2[:], rot_ps[:], sink_sb[:])
                    krn = abst.tile([ROPE, TSH], BF16, tag="krn")
                    nc.vector.tensor_add(krn[:], t1[:], t2[:])
                    nc.sync.dma_start(ag_kr_in[:], krn[:])

                    nc.gpsimd.collective_compute(
                        "AllGather", mybir.AluOpType.bypass,
                        replica_groups=[list(range(NCORES))],
                        ins=[ag_kv_in.opt()], outs=[ag_kv_out.opt()])
                    nc.gpsimd.collective_compute(
                        "AllGather", mybir.AluOpType.bypass,
                        replica_groups=[list(range(NCORES))],
                        ins=[ag_kr_in.opt()], outs=[ag_kr_out.opt()])

                    # ---------- q down-proj (12 chunks, 6 AG groups) -------
                    # RAW (unnormalized) chunks are AllGathered per group as
                    # soon as they finish; the RMS 1/sqrt factor is gathered
                    # separately and folded into the QT up-proj eviction.
                    sumsq_q = ss_ps.tile([1, TSH], F32, tag="ssq")
                    for g3 in range(3):
                        q_ps = [dp_ps.tile([128, TSH], F32, name=f"qps{m}",
                                           tag="dps") for m in range(4)]
                        for k in range(INC):
                            wt = abw.tile([128, 512], BF16, name="wqa_t",
                                          tag="wqa_t")
                            nc.scalar.dma_start(
                                wt[:], wqa[k, :, g3 * 512:(g3 + 1) * 512])
                            for m in range(4):
                                nc.tensor.matmul(
                                    q_ps[m][:], wt[:, m * 128:(m + 1) * 128],
                                    hidT[:, k, :],
                                    start=(k == 0), stop=(k == INC - 1))
                        for m in range(4):
                            mg = g3 * 4 + m
                            g = mg // 2
                            qr_t = abst.tile([128, TSH], BF16, name="qr_t",
                                             tag="qr_t")
                            nc.scalar.copy(qr_t[:], q_ps[m][:])
                            nc.sync.dma_start(
                                ag_q_in[g][(m % 2) * 128:(m % 2 + 1) * 128, :],
                                qr_t[:])
                            sq = abst.tile([128, TSH], F32R, name="sq", tag="sq")
                            nc.vector.tensor_mul(sq[:], qr_t[:], qr_t[:])
                            nc.tensor.matmul(sumsq_q[:], onesc_sb[:], sq[:],
                                             start=(mg == 0),
                                             stop=(mg == QRC - 1))
                            if mg == QRC - 1:
                                # inv-rms AG just before the last (small) q AG
                                s_q = abst.tile([1, TSH], F32, tag="s_small")
                                nc.scalar.activation(s_q[:], sumsq_q[:],
                                                     AF.Sqrt, bias=eps_sb[:],
                                                     scale=1.0 / QR)
                                inv_q = abst.tile([1, TSH], F32,
                                                  tag="inv_small")
                                nc.vector.reciprocal_approx_fast(inv_q[:],
                                                                 s_q[:])
                                nc.sync.dma_start(ag_iv_in[:], inv_q[:])
                                nc.gpsimd.collective_compute(
                                    "AllGather", mybir.AluOpType.bypass,
                                    replica_groups=[list(range(NCORES))],
                                    ins=[ag_iv_in.opt()],
                                    outs=[ag_iv_out.opt()])
                            if m % 2 == 1:
                                nc.gpsimd.collective_compute(
                                    "AllGather", mybir.AluOpType.bypass,
                                    replica_groups=[list(range(NCORES))],
                                    ins=[ag_q_in[g].opt()],
                                    outs=[ag_q_out[g].opt()])

            # ============ phase C: up-projections (head-parallel) ==========
            with tc.tile_pool(name="kn_sb", bufs=1) as kn_pool, \
                 tc.tile_pool(name="v_sb", bufs=1) as v_pool, \
                 tc.tile_pool(name="qt_sb", bufs=1) as qt_pool, \
                 tc.tile_pool(name="kr_sb", bufs=1) as kr_pool:

                knT = kn_pool.tile([128, HPC, TC, 512], F32R)   # 32 KB/part
                v_sb = v_pool.tile([128, TSH // 128 * NCORES, HPC * VHD], BF16)
                qT = qt_pool.tile([128, 2, TC, 512], F32R)      # 32 KB/part
                # roped q, bf16, per head doubled into both partition halves;
                # k_rope (halved on host via cos_k/sin_k) doubled the same way
                # so the rope score matmul contracts K=128 at full bf16 rate:
                # (kr/2).qr + (kr/2).qr = kr.qr
                qTr2 = qt_pool.tile([128, HPC, TC, 512], BF16)  # 16 KB/part
                krT = kr_pool.tile([2 * ROPE, TC, 512], BF16)
                nc.sync.dma_start(
                    krT[0:ROPE, :, :],
                    ag_kr_out.transpose([1, 0, 2]))
                nc.sync.dma_start(
                    krT[ROPE:, :, :],
                    ag_kr_out.transpose([1, 0, 2]))

                with tc.tile_pool(name="kvn_sb", bufs=8) as kvn_pool, \
                     tc.tile_pool(name="upw", bufs=1) as upw, \
                     tc.tile_pool(name="up_ps", bufs=6, space="PSUM") as up_ps:
                    wkn_sb = upw.tile([128, KVC, HPC * NOPE], BF16)
                    for k in range(KVC):
                        nc.scalar.dma_start(wkn_sb[:, k, :], wkvb_kn[k])
                    wv_sb = upw.tile([128, KVC, HPC * VHD], BF16)
                    for k in range(KVC):
                        nc.scalar.dma_start(wv_sb[:, k, :], wkvb_v[k])

                    # K_nope^T and V, streaming kv_norm^T tiles from the AG
                    for tcb in range(TC):
                        rh = []
                        for k in range(KVC):
                            rt = kvn_pool.tile([128, 512], BF16, name="kvn_t",
                                               tag="kvn_t")
                            nc.sync.dma_start(
                                rt[:],
                                ag_kv_out[tcb, k * 128:(k + 1) * 128, :])
                            rh.append(rt)
                        psn = [up_ps.tile([128, 512], F32, name=f"knps{hl}",
                                          tag="upps") for hl in range(HPC)]
                        for k in range(KVC):
                            for hl in range(HPC):
                                nc.tensor.matmul(
                                    psn[hl][:],
                                    wkn_sb[:, k, hl * NOPE:(hl + 1) * NOPE],
                                    rh[k][:],
                                    start=(k == 0), stop=(k == KVC - 1))
                        for hl in range(HPC):
                            nc.scalar.copy(knT[:, hl, tcb, :], psn[hl][:])
                        psv = [up_ps.tile([128, HPC * VHD], F32,
                                          name=f"vps{j}", tag="upps")
                               for j in range(4)]
                        for k in range(KVC):
                            for j in range(4):
                                nc.tensor.matmul(
                                    psv[j][:],
                                    rh[k][:, j * 128:(j + 1) * 128],
                                    wv_sb[:, k, :],
                                    start=(k == 0), stop=(k == KVC - 1))
                        for j in range(4):
                            nc.vector.tensor_copy(v_sb[:, tcb * 4 + j, :],
                                                  psv[j][:])

                # Q^T (3 chunks: nope h0, nope h1, rope doubled), with the
                # deferred RMS normalize folded into the PSUM eviction and
                # rope applied per token-chunk right after.
                with tc.tile_pool(name="agq_sb", bufs=6) as agq_pool, \
                     tc.tile_pool(name="qw_sb", bufs=1) as qw_pool, \
                     tc.tile_pool(name="rope_sb", bufs=2) as rope_pool, \
                     tc.tile_pool(name="ropest", bufs=2) as ropest, \
                     tc.tile_pool(name="qt_ps", bufs=4, space="PSUM") as qt_ps, \
                     tc.tile_pool(name="rr_ps", bufs=2, space="PSUM") as rr_ps:
                    wqb_sb = qw_pool.tile([128, QRC, HPC * QKD], BF16)
                    for k in range(QRC):
                        nc.scalar.dma_start(wqb_sb[:, k, :], wqb[k])
                    invq_sb = qw_pool.tile([1, TC, 512], F32)
                    nc.sync.dma_start(
                        invq_sb[:],
                        ag_iv_out.rearrange("r o t -> o r t"))
                    for tcb in range(TC):
                        ps = [qt_ps.tile([128, 512], F32, name=f"qtps{m}",
                                         tag="qtps") for m in range(3)]
                        for k in range(QRC):
                            rh16 = agq_pool.tile([128, 512], BF16, name="agq16",
                                                 tag="agq16")
                            nc.sync.dma_start(
                                rh16[:],
                                ag_q_out[k // 2][tcb,
                                                 (k % 2) * 128:(k % 2 + 1) * 128,
                                                 :])
                            for m in range(3):
                                nc.tensor.matmul(
                                    ps[m][:],
                                    wqb_sb[:, k, m * 128:(m + 1) * 128],
                                    rh16[:],
                                    start=(k == 0), stop=(k == QRC - 1))
                        # broadcast 1/rms across partitions on gpsimd, then
                        # evict with the normalize multiply
                        biq_sb = ropest.tile([128, 512], F32, name="biq_sb",
                                             tag="biq_sb")
                        nc.gpsimd.partition_broadcast(biq_sb[:],
                                                      invq_sb[:, tcb, :])
                        for m in range(2):
                            nc.vector.tensor_mul(qT[:, m, tcb, :], ps[m][:],
                                                 biq_sb[:])
                        qtmp = ropest.tile([128, 512], F32R, name="qtmp",
                                           tag="qtmp")
                        nc.vector.tensor_mul(qtmp[:], ps[2][:], biq_sb[:])
                        # q-rope on the rope chunk (layout [rope_h0; rope_h1])
                        cosq_t = rope_pool.tile([128, 512], F32, name="cosq_t",
                                                tag="cosq_t")
                        nc.sync.dma_start(cosq_t[:],
                                          cos_q[:, tcb * 512:(tcb + 1) * 512])
                        sinq_t = rope_pool.tile([128, 512], F32, name="sinq_t",
                                                tag="sinq_t")
                        nc.sync.dma_start(sinq_t[:],
                                          sin_q[:, tcb * 512:(tcb + 1) * 512])
                        rps = rr_ps.tile([128, 512], F32, tag="rrps")
                        nc.tensor.matmul(rps[:], r128_sb[:], qtmp[:],
                                         start=True, stop=True)
                        t1 = ropest.tile([128, 512], F32, name="rt1", tag="rt1")
                        nc.vector.tensor_mul(t1[:], qtmp[:], cosq_t[:])
                        t2 = ropest.tile([128, 512], F32, name="rt2", tag="rt2")
                        nc.vector.tensor_mul(t2[:], rps[:], sinq_t[:])
                        qrb = ropest.tile([128, 512], BF16, name="qrb",
                                          tag="qrb")
                        nc.vector.tensor_add(qrb[:], t1[:], t2[:])
                        # double each head's rope-q into both partition halves
                        for hl in range(HPC):
                            nc.sync.dma_start(
                                qTr2[0:ROPE, hl, tcb, :],
                                qrb[hl * ROPE:(hl + 1) * ROPE, :])
                            nc.sync.dma_start(
                                qTr2[ROPE:, hl, tcb, :],
                                qrb[hl * ROPE:(hl + 1) * ROPE, :])

                # ============ phase D: causal attention =================
                with tc.tile_pool(name="ao_sb", bufs=1) as ao_pool, \
                     tc.tile_pool(name="wo_sb", bufs=4) as wo_pool:
                  aosb = ao_pool.tile([128, KCO, 512], BF16)
                  with tc.tile_pool(name="pt_sb", bufs=6) as pt_pool, \
                     tc.tile_pool(name="att_st", bufs=2) as att_st, \
                     tc.tile_pool(name="st_ps", bufs=4, space="PSUM") as st_ps, \
                     tc.tile_pool(name="l_ps", bufs=2, space="PSUM") as l_ps, \
                     tc.tile_pool(name="o_ps", bufs=2, space="PSUM") as o_ps:
                    for hl in range(HPC):
                        for b in range(B):
                            for qc in range(SB):
                                tcq = b * SB + qc
                                nkb = 4 * qc + 4
                                op = o_ps.tile([128, 512], F32, name="op",
                                               tag="op")
                                lp = l_ps.tile([1, 512], F32, name="lp",
                                               tag="lp")

                                # software-pipelined: lp/op of block kb are
                                # emitted after the score matmuls of kb+1 so
                                # the PE never waits on the scalar-engine exp
                                def lp_op(kb, pt):
                                    nc.tensor.matmul(lp[:], onescb_sb[:],
                                                     pt[:],
                                                     start=(kb == 0),
                                                     stop=(kb == nkb - 1))
                                    nc.tensor.matmul(
                                        op[:],
                                        v_sb[:, b * NKB + kb,
                                             hl * VHD:(hl + 1) * VHD],
                                        pt[:],
                                        start=(kb == 0), stop=(kb == nkb - 1))

                                prev = None
                                for kb in range(nkb):
                                    tck = b * SB + kb // 4
                                    co = (kb % 4) * 128
                                    st = st_ps.tile([128, 512], F32,
                                                    name="st", tag="st")
                                    diag = kb >= 4 * qc
                                    nc.tensor.matmul(
                                        st[:],
                                        knT[:, hl, tck, co:co + 128],
                                        qT[:, hl, tcq, :],
                                        start=True, stop=False)
                                    nc.tensor.matmul(
                                        st[:],
                                        krT[:, tck, co:co + 128],
                                        qTr2[:, hl, tcq, :],
                                        start=False, stop=True)
                                    if prev is not None:
                                        lp_op(*prev)
                                    if diag:
                                        nc.vector.tensor_add(
                                            st[:], st[:],
                                            maskb_sb[:, kb - 4 * qc, :])
                                    pt = pt_pool.tile([128, 512], BF16,
                                                      name="pt", tag="pt")
                                    nc.scalar.activation(pt[:], st[:], AF.Exp,
                                                         scale=SCALE)
                                    prev = (kb, pt)
                                lp_op(*prev)
                                invl = att_st.tile([1, 512], F32, name="invl",
                                                   tag="invl")
                                nc.vector.reciprocal_approx_fast(invl[:], lp[:])
                                bi = att_st.tile([128, 512], F32, name="bi",
                                                 tag="bi")
                                nc.gpsimd.partition_broadcast(bi[:], invl[:])
                                att = att_st.tile([128, 512], BF16, name="att",
                                                  tag="att")
                                nc.vector.tensor_mul(att[:], op[:], bi[:])
                                nc.sync.dma_start(
                                    a2a_in[hl][tcq, :, :], att[:])
                        nc.gpsimd.collective_compute(
                            "AllToAll", mybir.AluOpType.bypass,
                            replica_groups=[list(range(NCORES))],
                            ins=[a2a_in[hl].opt()], outs=[a2a_out[hl].opt()])
                        for i in range(NCORES):
                            nc.sync.dma_start(aosb[:, 2 * i + hl, :],
                                              a2a_out[hl][i])

                  # ============ phase E: o_proj (token-parallel) ===========
                  # Two passes: head-0 contraction chunks (depend only on the
                  # first AllToAll) accumulate + evict partials; the head-1
                  # pass re-seeds PSUM from the partials via identity matmul.
                  with tc.tile_pool(name="yp_sb", bufs=1) as yp_pool, \
                       tc.tile_pool(name="yo_sb", bufs=3) as yo_pool, \
                       tc.tile_pool(name="op_ps", bufs=8, space="PSUM") as op_ps:
                    yp = yp_pool.tile([128, HID // 512, 4, 512], F32R)
                    kc_h0 = [2 * i for i in range(NCORES)]
                    kc_h1 = [2 * i + 1 for i in range(NCORES)]
                    for n in range(HID // 512):
                        pso = [op_ps.tile([128, 512], F32, name=f"pso{mt}",
                                          tag="pso") for mt in range(4)]
                        for ki, kc in enumerate(kc_h0):
                            wot = wo_pool.tile([128, 512], BF16, name="wot",
                                               tag="wot")
                            nc.scalar.dma_start(
                                wot[:], wo[kc, :, n * 512:(n + 1) * 512])
                            for mt in range(4):
                                nc.tensor.matmul(
                                    pso[mt][:],
                                    aosb[:, kc, mt * 128:(mt + 1) * 128],
                                    wot[:],
                                    start=(ki == 0), stop=(ki == NCORES - 1))
                        for mt in range(4):
                            nc.scalar.copy(yp[:, n, mt, :], pso[mt][:])
                    for n in range(HID // 512):
                        pso = [op_ps.tile([128, 512], F32, name=f"ps2{mt}",
                                          tag="pso") for mt in range(4)]
                        for mt in range(4):
                            nc.tensor.matmul(pso[mt][:], identr_sb[:],
                                             yp[:, n, mt, :],
                                             start=True, stop=False)
                        for ki, kc in enumerate(kc_h1):
                            wot = wo_pool.tile([128, 512], BF16, name="wot",
                                               tag="wot")
                            nc.scalar.dma_start(
                                wot[:], wo[kc, :, n * 512:(n + 1) * 512])
                            for mt in range(4):
                                nc.tensor.matmul(
                                    pso[mt][:],
                                    aosb[:, kc, mt * 128:(mt + 1) * 128],
                                    wot[:],
                                    start=False, stop=(ki == NCORES - 1))
                        for mt in range(4):
                            yt = yo_pool.tile([128, 512], F32, name="yt",
                                              tag="yt")
                            nc.scalar.copy(yt[:], pso[mt][:])
                            nc.sync.dma_start(
                                y[mt * 128:(mt + 1) * 128,
                                  n * 512:(n + 1) * 512], yt[:])

    nc.compile()
    return nc


def _rot_lhsT(n):
    """lhsT for the interleaved rotate-half as a matmul: out = R @ x,
    R[2i, 2i+1] = -1, R[2i+1, 2i] = +1; matmul computes lhsT.T @ rhs."""
    R = np.zeros((n, n), dtype=np.float32)
    for i in range(n // 2):
        R[2 * i, 2 * i + 1] = -1.0
        R[2 * i + 1, 2 * i] = 1.0
    return np.ascontiguousarray(R.T)


def _prep_inputs(inputs):
    """Host-side sharding/reordering. Returns in_maps (list of 8 dicts)."""
    import ml_dtypes
    bf16 = ml_dtypes.bfloat16

    hs = np.ascontiguousarray(
        np.asarray(inputs["hidden_states"], dtype=np.float32).reshape(T, IN))
    Wq_a = np.asarray(inputs["Wq_a"], dtype=np.float32)
    q_a_ln = np.asarray(inputs["q_a_ln"], dtype=np.float32)
    Wq_b = np.asarray(inputs["Wq_b"], dtype=np.float32) * q_a_ln[:, None]
    Wkv_a = np.asarray(inputs["Wkv_a"], dtype=np.float32)
    kv_a_ln = np.asarray(inputs["kv_a_ln"], dtype=np.float32)
    Wkv_b = np.asarray(inputs["Wkv_b"], dtype=np.float32) * kv_a_ln[:, None]
    Wo = np.asarray(inputs["Wo"], dtype=np.float32)
    pos = np.asarray(inputs["position_ids"]).astype(np.float64)   # [B, S]

    # rope tables (doubled pairs): cos[2i] = cos[2i+1] = cos(pos * invf_i)
    invf = 1.0 / (THETA ** (np.arange(0, ROPE, 2, dtype=np.float64) / ROPE))
    fr = pos[..., None] * invf                       # [B, S, 32]
    cosd = np.repeat(np.cos(fr), 2, axis=-1).astype(np.float32)  # [B, S, 64]
    sind = np.repeat(np.sin(fr), 2, axis=-1).astype(np.float32)
    cosT = np.ascontiguousarray(cosd.reshape(T, ROPE).T)   # [64, T]
    sinT = np.ascontiguousarray(sind.reshape(T, ROPE).T)
    cos_q = np.concatenate([cosT, cosT], axis=0)           # [128, T]
    sin_q = np.concatenate([sinT, sinT], axis=0)

    # causal additive mask biases for diagonal blocks
    maskb = np.zeros((4, 128, 512), dtype=np.float32)
    kl = np.arange(128)[:, None]
    ql = np.arange(512)[None, :]
    for r in range(4):
        maskb[r] = np.where(kl + 128 * r > ql, NEG, 0.0)

    ident = np.eye(128, dtype=np.float32)
    onesc = np.ones((128, 1), dtype=np.float32)
    onesr = np.ones((1, 128), dtype=np.float32)

    wqa_r = np.ascontiguousarray(
        Wq_a.reshape(IN // 128, 128, QR).astype(bf16))
    wkva_r = np.ascontiguousarray(
        Wkv_a.reshape(IN // 128, 128, KVR + ROPE).astype(bf16))
    wo_r = np.ascontiguousarray(
        Wo.reshape(H * VHD // 128, 128, HID).astype(bf16))

    Wq_b_h = Wq_b.reshape(QR, H, QKD)
    Wkv_b_h = Wkv_b.reshape(KVR, H, NOPE + VHD)

    in_maps = []
    for c in range(NCORES):
        h0, h1 = HPC * c, HPC * c + 1
        bc = c // (NCORES // B)
        s0 = (c % (NCORES // B)) * TSH
        # reorder q_b cols: [nope_h0 | nope_h1 | rope_h0 ; rope_h1]
        wqb_s = np.concatenate([
            Wq_b_h[:, h0, :NOPE], Wq_b_h[:, h1, :NOPE],
            Wq_b_h[:, h0, NOPE:], Wq_b_h[:, h1, NOPE:]], axis=1)
        wqb_s = np.ascontiguousarray(
            wqb_s.reshape(QR // 128, 128, HPC * QKD).astype(bf16))
        wkvb_kn_s = np.ascontiguousarray(
            np.concatenate([Wkv_b_h[:, h0, :NOPE], Wkv_b_h[:, h1, :NOPE]],
                           axis=1).reshape(KVR // 128, 128, HPC * NOPE)
            .astype(bf16))
        wkvb_v_s = np.ascontiguousarray(
            np.concatenate([Wkv_b_h[:, h0, NOPE:], Wkv_b_h[:, h1, NOPE:]],
                           axis=1).reshape(KVR // 128, 128, HPC * VHD)
            .astype(bf16))
        tok0 = c * TSH
        in_maps.append({
            "hid": np.ascontiguousarray(hs[tok0:tok0 + TSH]),
            "wqa": wqa_r, "wqb": wqb_s, "wkva": wkva_r,
            "wkvb_kn": wkvb_kn_s, "wkvb_v": wkvb_v_s, "wo": wo_r,
            # k-side tables are halved: the rope score matmul contracts the
            # per-head-doubled layout, summing the halved product twice
            "cos_k": np.ascontiguousarray(
                0.5 * cosT[:, bc * S + s0: bc * S + s0 + TSH]),
            "sin_k": np.ascontiguousarray(
                0.5 * sinT[:, bc * S + s0: bc * S + s0 + TSH]),
            "cos_q": cos_q, "sin_q": sin_q,
            "maskb": maskb, "identr": ident,
            "r128": np.ascontiguousarray(
                np.block([[_rot_lhsT(ROPE), np.zeros((ROPE, ROPE), np.float32)],
                          [np.zeros((ROPE, ROPE), np.float32), _rot_lhsT(ROPE)]])),
            "r64": _rot_lhsT(ROPE),
            "onesc": onesc, "onescb": onesc.astype(bf16), "onesr": onesr,
        })
    return in_maps


def kernel(**inputs) -> np.ndarray:
    from concourse.bass_utils import run_bass_kernel_spmd

    if "nc" not in _cache:
        _cache["nc"] = _build()
    nc = _cache["nc"]
    in_maps = _prep_inputs(inputs)
    res = run_bass_kernel_spmd(nc, in_maps, core_ids=list(range(NCORES)))
    out = np.concatenate([res.results[c]["y"] for c in range(NCORES)], axis=0)
    return np.ascontiguousarray(out.reshape(B, S, HID))


if __name__ == "__main__":
    rng = np.random.default_rng(0)
    ins = {
        "hidden_states": rng.standard_normal((B, S, IN), dtype=np.float32),
        "Wq_a": rng.standard_normal((IN, QR), dtype=np.float32) * IN ** -0.5,
        "q_a_ln": np.ones(QR, np.float32),
        "Wq_b": rng.standard_normal((QR, H * QKD), dtype=np.float32) * QR ** -0.5,
        "Wkv_a": rng.standard_normal((IN, KVR + ROPE), dtype=np.float32) * IN ** -0.5,
        "kv_a_ln": np.ones(KVR, np.float32),
        "Wkv_b": rng.standard_normal((KVR, H * (NOPE + VHD)), dtype=np.float32) * KVR ** -0.5,
        "Wo": rng.standard_normal((H * VHD, HID), dtype=np.float32) * (H * VHD) ** -0.5,
        "position_ids": np.tile(np.arange(S, dtype=np.int32)[None], (B, 1)),
    }
    out = kernel(**ins)
    print("kernel ran, out shape", out.shape, "absmax", np.abs(out).max())


# revision 34
# speedup vs baseline: 1.0671x; 1.0392x over previous
"""DeepSeek-MLA attention Trainium2 Bass kernel, 8-core SPMD.

Sharding strategy (one NEFF, per-core data differs):
  - Tokens (B*S = 4096) are sharded 512/core for the down-projections and o_proj.
  - Heads (16) are sharded 2/core for the up-projections and attention.
  - Device collectives stitch the two shardings together:
      AllGather(kv_norm^T bf16 + k_rope^T f32r)  after the joint kv down-proj,
      AllGather(q_lora^T bf16) x3                after the q down-proj,
      AllToAll(attn_out^T bf16)                  head-parallel -> token-parallel
  - Projection matmuls run in bf16 (weights cast on host); the attention
    score matmuls (q.k) run in f32r to keep softmax logits accurate.
  - Dataflow is kept feature-major ("T layout": [feature, token]) so no
    transposes are needed anywhere except hidden_states itself (PE transpose).
  - Causal softmax without a running max (scores are O(+-5) after scale, exp
    cannot overflow); masking is an additive -1e9 applied into the score PSUM
    by the vector engine; the denominator l = sum_k P is accumulated on the
    vector engine and partition-reduced on gpsimd; 1/l via the fast DVE
    reciprocal and broadcast across partitions on gpsimd. This keeps the
    tensor engine stream free of bookkeeping matmuls.
  - o_proj is split into a head-0 half (depends only on the first AllToAll)
    and a head-1 half re-seeded from the evicted partials via an identity
    matmul, so the in-order PE queue never stalls on the second AllToAll.

RMSNorm weights are folded into the up-projection weights on the host
(host does only O(d^2) reshuffles + dtype casts; all O(n^3) math on device).
"""

import math

import numpy as np

# ---- problem shapes (hardcoded; harness contract) ----
B, S, HID = 2, 2048, 2048
IN = 2 * HID
H = 16
NOPE, ROPE, VHD = 128, 64, 128
QKD = NOPE + ROPE
QR, KVR = 1536, 512
EPS = 1e-6
THETA = 10000.0
SCALE = 1.0 / math.sqrt(QKD)

NCORES = 8
T = B * S                 # 4096 flat tokens (b-major)
TSH = T // NCORES         # 512 tokens per core
HPC = H // NCORES         # 2 heads per core
NEG = -1.0e9

_cache = {}


def _build():
    import concourse.bass as bass
    import concourse.mybir as mybir
    import concourse.tile as tile
    from concourse import bacc

    dt = mybir.dt
    F32 = dt.float32
    F32R = dt.float32r
    BF16 = dt.bfloat16
    AF = mybir.ActivationFunctionType
    AX = mybir.AxisListType
    ALU = mybir.AluOpType

    nc = bacc.Bacc("TRN2", target_bir_lowering=False, debug=False,
                   num_devices=NCORES)

    # ---------------- I/O ----------------
    def inp(name, shape, dtype=F32R):
        return nc.dram_tensor(name, shape, dtype, kind="ExternalInput").ap()

    hid = inp("hid", [TSH, IN], BF16)                 # natural token shard
    wqa = inp("wqa", [IN // 128, 128, QR], BF16)      # full
    wqb = inp("wqb", [QR // 128, 128, HPC * QKD], BF16)   # shard, reordered
    wkva = inp("wkva", [IN // 128, 128, KVR + ROPE], BF16)  # full
    wkvb_kn = inp("wkvb_kn", [KVR // 128, 128, HPC * NOPE], BF16)
    wkvb_v = inp("wkvb_v", [KVR // 128, 128, HPC * VHD], BF16)
    wo = inp("wo", [H * VHD // 128, 128, HID], BF16)  # full
    cos_k = inp("cos_k", [ROPE, TSH], F32)
    sin_k = inp("sin_k", [ROPE, TSH], F32)
    cos_q = inp("cos_q", [2 * ROPE, T], F32)          # doubled for 2 heads
    sin_q = inp("sin_q", [2 * ROPE, T], F32)
    maskb = inp("maskb", [4, 128, 512], F32)          # additive causal biases
    identr = inp("identr", [128, 128])                # f32r identity
    identb = inp("identb", [128, 128], BF16)          # bf16 identity
    onesc = inp("onesc", [128, 1])
    onescb = inp("onescb", [128, 1], BF16)
    onesr = inp("onesr", [1, 128])
    r128 = inp("r128", [128, 128])                    # q-rope rotation lhsT
    r64 = inp("r64", [ROPE, ROPE])                    # k-rope rotation lhsT

    y = nc.dram_tensor("y", [TSH, HID], F32, kind="ExternalOutput").ap()

    QRC = QR // 128            # 12 q-lora chunks
    KVC = KVR // 128           # 4 kv chunks
    INC = IN // 128            # 32 input chunks
    TC = T // 512              # 8 token chunks (flat)
    SB = S // 512              # 4 token chunks per batch
    NKB = S // 128             # 16 key blocks per batch
    KCO = H * VHD // 128       # 16 o_proj contraction chunks

    with tile.TileContext(nc) as tc:
        with tc.tile_pool(name="dram", bufs=1, space="DRAM") as dram, \
             tc.tile_pool(name="const", bufs=1) as const:

            # ---- DRAM bounce buffers for collectives ----
            ag_kv_in = dram.tile([KVR, TSH], BF16)
            ag_kv_out = dram.tile([NCORES, KVR, TSH], BF16,
                                  addr_space="Shared")
            ag_kr_in = dram.tile([ROPE, TSH], BF16, name="ag_kr_in")
            ag_kr_out = dram.tile([NCORES, ROPE, TSH], BF16,
                                  addr_space="Shared", name="ag_kr_out")
            ag_q_in = dram.tile([QR, TSH], BF16, name="ag_q_in")
            ag_q_out = dram.tile([NCORES, QR, TSH], BF16,
                                 addr_space="Shared", name="ag_q_out")
            ag_iv_in = dram.tile([1, TSH], F32)
            ag_iv_out = dram.tile([NCORES, 1, TSH], F32, addr_space="Shared")
            a2a_in = [dram.tile([NCORES, VHD, TSH], BF16, name=f"a2a_in{hl}")
                      for hl in range(HPC)]
            a2a_out = [dram.tile([NCORES, VHD, TSH], BF16,
                                 name=f"a2a_out{hl}")
                       for hl in range(HPC)]

            # ---- small constants resident in SBUF ----
            identr_sb = const.tile([128, 128], F32R)
            nc.sync.dma_start(identr_sb[:], identr[:])
            identb_sb = const.tile([128, 128], BF16)
            nc.sync.dma_start(identb_sb[:], identb[:])
            r128_sb = const.tile([128, 128], F32R)
            nc.sync.dma_start(r128_sb[:], r128[:])
            r64_sb = const.tile([ROPE, ROPE], F32R)
            nc.sync.dma_start(r64_sb[:], r64[:])
            onesc_sb = const.tile([128, 1], F32R)
            nc.sync.dma_start(onesc_sb[:], onesc[:])
            onescb_sb = const.tile([128, 1], BF16)
            nc.sync.dma_start(onescb_sb[:], onescb[:])
            onesr_sb = const.tile([1, 128], F32R)
            nc.sync.dma_start(onesr_sb[:], onesr[:])
            cosk_sb = const.tile([ROPE, TSH], F32)
            nc.sync.dma_start(cosk_sb[:], cos_k[:])
            sink_sb = const.tile([ROPE, TSH], F32)
            nc.sync.dma_start(sink_sb[:], sin_k[:])
            maskb_sb = const.tile([128, 4, 512], F32)
            for r in range(4):
                nc.sync.dma_start(maskb_sb[:, r, :], maskb[r])
            eps_sb = const.tile([1, 1], F32)
            nc.vector.memset(eps_sb[:], EPS)

            # ================= phase A/B: hidden^T, down-proj, AGs ========
            with tc.tile_pool(name="ab_sbuf", bufs=1) as ab, \
                 tc.tile_pool(name="ab_w", bufs=12) as abw, \
                 tc.tile_pool(name="ab_stage", bufs=3) as abst:

                # ---- hidden^T via PE transposes (bf16) ----
                # 16 column-sliced tiles issued column-major across two IO
                # queues: the first transposes start after ~0.25MB of DMA
                hidT = ab.tile([128, INC, TSH], BF16)   # 32 KB/part
                with tc.tile_pool(name="hidn_p", bufs=16) as hidn_p, \
                     tc.tile_pool(name="tp_ps", bufs=3, space="PSUM") as tp_ps:
                    hidn = {}
                    for qcol in range(4):
                        for tb in range(TSH // 128):
                            ht = hidn_p.tile([128, 1024], BF16, name="hidn",
                                             tag="hidn")
                            eng = nc.sync if (qcol * 4 + tb) % 2 else nc.scalar
                            eng.dma_start(
                                ht[:],
                                hid[tb * 128:(tb + 1) * 128,
                                    qcol * 1024:(qcol + 1) * 1024])
                            hidn[tb, qcol] = ht
                    for icg_g in range(INC // 4):
                        qcol = icg_g // 2
                        for tb in range(TSH // 128):
                            tps = tp_ps.tile([128, 512], BF16, name="tps",
                                             tag="tps")
                            for j in range(4):
                                co = (icg_g * 4 + j) * 128 - qcol * 1024
                                nc.tensor.transpose(
                                    tps[:, j * 128:(j + 1) * 128],
                                    hidn[tb, qcol][:, co:co + 128],
                                    identb_sb[:])
                            nc.scalar.copy(
                                hidT[:, icg_g * 4:(icg_g + 1) * 4,
                                     tb * 128:(tb + 1) * 128],
                                tps.rearrange("p (c t) -> p c t", t=128))

                with tc.tile_pool(name="dp_ps", bufs=6, space="PSUM") as dp_ps, \
                     tc.tile_pool(name="ss_ps", bufs=1, space="PSUM") as ss_ps, \
                     tc.tile_pool(name="ms_ps", bufs=1, space="PSUM") as ms_ps:

                    # ---------- kv down-proj (5 out chunks: 4 kv + rope) ----
                    kv_ps = [dp_ps.tile([128, TSH], F32, name=f"kvps{m}",
                                        tag="dps") for m in range(KVC)]
                    kr_ps = dp_ps.tile([ROPE, TSH], F32, tag="dps")
                    for k in range(INC):
                        wt = abw.tile([128, KVR + ROPE], BF16, name="wkva_t",
                                      tag="wkva_t")
                        nc.scalar.dma_start(wt[:], wkva[k])
                        for m in range(KVC):
                            nc.tensor.matmul(
                                kv_ps[m][:], wt[:, m * 128:(m + 1) * 128],
                                hidT[:, k, :], start=(k == 0), stop=(k == INC - 1))
                        nc.tensor.matmul(
                            kr_ps[:], wt[:, KVR:], hidT[:, k, :],
                            start=(k == 0), stop=(k == INC - 1))

                    # rms over kv chunks
                    kv_raw = [ab.tile([128, TSH], F32, name=f"kvraw{m}",
                                      tag=f"kvraw{m}") for m in range(KVC)]
                    sumsq_kv = ss_ps.tile([1, TSH], F32, tag="ssq")
                    for m in range(KVC):
                        nc.scalar.copy(kv_raw[m][:], kv_ps[m][:])
                        sq = abst.tile([128, TSH], F32R, name="sq", tag="sq")
                        nc.vector.tensor_mul(sq[:], kv_raw[m][:], kv_raw[m][:])
                        nc.tensor.matmul(sumsq_kv[:], onesc_sb[:], sq[:],
                                         start=(m == 0), stop=(m == KVC - 1))
                    s_kv = abst.tile([1, TSH], F32, tag="s_small")
                    nc.scalar.activation(s_kv[:], sumsq_kv[:], AF.Sqrt,
                                         bias=eps_sb[:], scale=1.0 / KVR)
                    inv_kv = abst.tile([1, TSH], F32, tag="inv_small")
                    nc.vector.reciprocal_approx_fast(inv_kv[:], s_kv[:])
                    binv = abst.tile([128, TSH], F32, tag="binv")
                    nc.gpsimd.partition_broadcast(binv[:], inv_kv[:])
                    for m in range(KVC):
                        kvn = abst.tile([128, TSH], BF16, name="kvn", tag="kvn")
                        nc.vector.tensor_mul(kvn[:], kv_raw[m][:], binv[:])
                        nc.sync.dma_start(
                            ag_kv_in[m * 128:(m + 1) * 128, :], kvn[:])

                    # k-rope: rotate + cos/sin (token shard only)
                    krope_raw = ab.tile([ROPE, TSH], F32R)
                    nc.scalar.copy(krope_raw[:], kr_ps[:])
                    rot_ps = ms_ps.tile([ROPE, TSH], F32, tag="msps")
                    nc.tensor.matmul(rot_ps[:], r64_sb[:], krope_raw[:],
                                     start=True, stop=True)
                    t1 = abst.tile([ROPE, TSH], F32, tag="ropet1")
                    nc.vector.tensor_mul(t1[:], krope_raw[:], cosk_sb[:])
                    t2 = abst.tile([ROPE, TSH], F32, tag="ropet2")
                    nc.vector.tensor_mul(t2[:], rot_ps[:], sink_sb[:])
                    krn = abst.tile([ROPE, TSH], BF16, tag="krn")
                    nc.vector.tensor_add(krn[:], t1[:], t2[:])
                    nc.sync.dma_start(ag_kr_in[:], krn[:])

                    nc.gpsimd.collective_compute(
                        "AllGather", mybir.AluOpType.bypass,
                        replica_groups=[list(range(NCORES))],
                        ins=[ag_kv_in.opt()], outs=[ag_kv_out.opt()])
                    nc.gpsimd.collective_compute(
                        "AllGather", mybir.AluOpType.bypass,
                        replica_groups=[list(range(NCORES))],
                        ins=[ag_kr_in.opt()], outs=[ag_kr_out.opt()])

                    # ---------- q down-proj (12 chunks, 6 AG groups) -------
                    # RAW (unnormalized) chunks are AllGathered per group as
                    # soon as they finish; the RMS 1/sqrt factor is gathered
                    # separately and folded into the QT up-proj eviction.
                    sumsq_q = ss_ps.tile([1, TSH], F32, tag="ssq")
                    for g3 in range(3):
                        q_ps = [dp_ps.tile([128, TSH], F32, name=f"qps{m}",
                                           tag="dps") for m in range(4)]
                        for k in range(INC):
                            wt = abw.tile([128, 512], BF16, name="wqa_t",
                                          tag="wqa_t")
                            nc.scalar.dma_start(
                                wt[:], wqa[k, :, g3 * 512:(g3 + 1) * 512])
                            for m in range(4):
                                nc.tensor.matmul(
                                    q_ps[m][:], wt[:, m * 128:(m + 1) * 128],
                                    hidT[:, k, :],
                                    start=(k == 0), stop=(k == INC - 1))
                        for m in range(4):
                            mg = g3 * 4 + m
                            qr_t = abst.tile([128, TSH], BF16, name="qr_t",
                                             tag="qr_t")
                            nc.scalar.copy(qr_t[:], q_ps[m][:])
                            nc.sync.dma_start(
                                ag_q_in[mg * 128:(mg + 1) * 128, :], qr_t[:])
                            sq = abst.tile([128, TSH], F32R, name="sq", tag="sq")
                            nc.vector.tensor_mul(sq[:], qr_t[:], qr_t[:])
                            nc.tensor.matmul(sumsq_q[:], onesc_sb[:], sq[:],
                                             start=(mg == 0),
                                             stop=(mg == QRC - 1))
                            if mg == QRC - 1:
                                s_q = abst.tile([1, TSH], F32, tag="s_small")
                                nc.scalar.activation(s_q[:], sumsq_q[:],
                                                     AF.Sqrt, bias=eps_sb[:],
                                                     scale=1.0 / QR)
                                inv_q = abst.tile([1, TSH], F32,
                                                  tag="inv_small")
                                nc.vector.reciprocal_approx_fast(inv_q[:],
                                                                 s_q[:])
                                nc.sync.dma_start(ag_iv_in[:], inv_q[:])
                                nc.gpsimd.collective_compute(
                                    "AllGather", mybir.AluOpType.bypass,
                                    replica_groups=[list(range(NCORES))],
                                    ins=[ag_iv_in.opt()],
                                    outs=[ag_iv_out.opt()])
                                nc.gpsimd.collective_compute(
                                    "AllGather", mybir.AluOpType.bypass,
                                    replica_groups=[list(range(NCORES))],
                                    ins=[ag_q_in.opt()],
                                    outs=[ag_q_out.opt()])

            # ============ phase C: up-projections (head-parallel) ==========
            with tc.tile_pool(name="kn_sb", bufs=1) as kn_pool, \
                 tc.tile_pool(name="v_sb", bufs=1) as v_pool, \
                 tc.tile_pool(name="qt_sb", bufs=1) as qt_pool, \
                 tc.tile_pool(name="kr_sb", bufs=1) as kr_pool:

                knT = kn_pool.tile([128, HPC, TC, 512], F32R)   # 32 KB/part
                v_sb = v_pool.tile([128, TSH // 128 * NCORES, HPC * VHD], BF16)
                qT = qt_pool.tile([128, 2, TC, 512], F32R)      # 32 KB/part
                # roped q, bf16, per head doubled into both partition halves;
                # k_rope (halved on host via cos_k/sin_k) doubled the same way
                # so the rope score matmul contracts K=128 at full bf16 rate:
                # (kr/2).qr + (kr/2).qr = kr.qr
                qTr2 = qt_pool.tile([128, HPC, TC, 512], BF16)  # 16 KB/part
                krT = kr_pool.tile([2 * ROPE, TC, 512], BF16)
                nc.sync.dma_start(
                    krT[0:ROPE, :, :],
                    ag_kr_out.transpose([1, 0, 2]))
                nc.sync.dma_start(
                    krT[ROPE:, :, :],
                    ag_kr_out.transpose([1, 0, 2]))

                with tc.tile_pool(name="kvn_sb", bufs=8) as kvn_pool, \
                     tc.tile_pool(name="upw", bufs=1) as upw, \
                     tc.tile_pool(name="up_ps", bufs=6, space="PSUM") as up_ps:
                    wkn_sb = upw.tile([128, KVC, HPC * NOPE], BF16)
                    for k in range(KVC):
                        nc.scalar.dma_start(wkn_sb[:, k, :], wkvb_kn[k])
                    wv_sb = upw.tile([128, KVC, HPC * VHD], BF16)
                    for k in range(KVC):
                        nc.scalar.dma_start(wv_sb[:, k, :], wkvb_v[k])

                    # K_nope^T and V, streaming kv_norm^T tiles from the AG
                    for tcb in range(TC):
                        rh = []
                        for k in range(KVC):
                            rt = kvn_pool.tile([128, 512], BF16, name="kvn_t",
                                               tag="kvn_t")
                            nc.sync.dma_start(
                                rt[:],
                                ag_kv_out[tcb, k * 128:(k + 1) * 128, :])
                            rh.append(rt)
                        psn = [up_ps.tile([128, 512], F32, name=f"knps{hl}",
                                          tag="upps") for hl in range(HPC)]
                        for k in range(KVC):
                            for hl in range(HPC):
                                nc.tensor.matmul(
                                    psn[hl][:],
                                    wkn_sb[:, k, hl * NOPE:(hl + 1) * NOPE],
                                    rh[k][:],
                                    start=(k == 0), stop=(k == KVC - 1))
                        for hl in range(HPC):
                            nc.scalar.copy(knT[:, hl, tcb, :], psn[hl][:])
                        psv = [up_ps.tile([128, HPC * VHD], F32,
                                          name=f"vps{j}", tag="upps")
                               for j in range(4)]
                        for k in range(KVC):
                            for j in range(4):
                                nc.tensor.matmul(
                                    psv[j][:],
                                    rh[k][:, j * 128:(j + 1) * 128],
                                    wv_sb[:, k, :],
                                    start=(k == 0), stop=(k == KVC - 1))
                        for j in range(4):
                            nc.vector.tensor_copy(v_sb[:, tcb * 4 + j, :],
                                                  psv[j][:])

                # Q^T (3 chunks: nope h0, nope h1, rope doubled), with the
                # deferred RMS normalize folded into the PSUM eviction and
                # rope applied per token-chunk right after.
                with tc.tile_pool(name="agq_sb", bufs=6) as agq_pool, \
                     tc.tile_pool(name="qw_sb", bufs=1) as qw_pool, \
                     tc.tile_pool(name="rope_sb", bufs=2) as rope_pool, \
                     tc.tile_pool(name="ropest", bufs=2) as ropest, \
                     tc.tile_pool(name="qt_ps", bufs=4, space="PSUM") as qt_ps, \
                     tc.tile_pool(name="rr_ps", bufs=2, space="PSUM") as rr_ps:
                    wqb_sb = qw_pool.tile([128, QRC, HPC * QKD], BF16)
                    for k in range(QRC):
                        nc.scalar.dma_start(wqb_sb[:, k, :], wqb[k])
                    invq_sb = qw_pool.tile([1, TC, 512], F32)
                    nc.sync.dma_start(
                        invq_sb[:],
                        ag_iv_out.rearrange("r o t -> o r t"))
                    for tcb in range(TC):
                        ps = [qt_ps.tile([128, 512], F32, name=f"qtps{m}",
                                         tag="qtps") for m in range(3)]
                        for k in range(QRC):
                            rh16 = agq_pool.tile([128, 512], BF16, name="agq16",
                                                 tag="agq16")
                            nc.sync.dma_start(
                                rh16[:],
                                ag_q_out[tcb, k * 128:(k + 1) * 128, :])
                            for m in range(3):
                                nc.tensor.matmul(
                                    ps[m][:],
                                    wqb_sb[:, k, m * 128:(m + 1) * 128],
                                    rh16[:],
                                    start=(k == 0), stop=(k == QRC - 1))
                        # broadcast 1/rms across partitions on gpsimd, then
                        # evict with the normalize multiply
                        biq_sb = ropest.tile([128, 512], F32, name="biq_sb",
                                             tag="biq_sb")
                        nc.gpsimd.partition_broadcast(biq_sb[:],
                                                      invq_sb[:, tcb, :])
                        for m in range(2):
                            nc.vector.tensor_mul(qT[:, m, tcb, :], ps[m][:],
                                                 biq_sb[:])
                        qtmp = ropest.tile([128, 512], F32R, name="qtmp",
                                           tag="qtmp")
                        nc.vector.tensor_mul(qtmp[:], ps[2][:], biq_sb[:])
                        # q-rope on the rope chunk (layout [rope_h0; rope_h1])
                        cosq_t = rope_pool.tile([128, 512], F32, name="cosq_t",
                                                tag="cosq_t")
                        nc.sync.dma_start(cosq_t[:],
                                          cos_q[:, tcb * 512:(tcb + 1) * 512])
                        sinq_t = rope_pool.tile([128, 512], F32, name="sinq_t",
                                                tag="sinq_t")
                        nc.sync.dma_start(sinq_t[:],
                                          sin_q[:, tcb * 512:(tcb + 1) * 512])
                        rps = rr_ps.tile([128, 512], F32, tag="rrps")
                        nc.tensor.matmul(rps[:], r128_sb[:], qtmp[:],
                                         start=True, stop=True)
                        t1 = ropest.tile([128, 512], F32, name="rt1", tag="rt1")
                        nc.vector.tensor_mul(t1[:], qtmp[:], cosq_t[:])
                        t2 = ropest.tile([128, 512], F32, name="rt2", tag="rt2")
                        nc.vector.tensor_mul(t2[:], rps[:], sinq_t[:])
                        qrb = ropest.tile([128, 512], BF16, name="qrb",
                                          tag="qrb")
                        nc.vector.tensor_add(qrb[:], t1[:], t2[:])
                        # double each head's rope-q into both partition halves
                        for hl in range(HPC):
                            nc.sync.dma_start(
                                qTr2[0:ROPE, hl, tcb, :],
                                qrb[hl * ROPE:(hl + 1) * ROPE, :])
                            nc.sync.dma_start(
                                qTr2[ROPE:, hl, tcb, :],
                                qrb[hl * ROPE:(hl + 1) * ROPE, :])

                # ============ phase D: causal attention =================
                with tc.tile_pool(name="ao_sb", bufs=1) as ao_pool, \
                     tc.tile_pool(name="wo_sb", bufs=4) as wo_pool:
                  aosb = ao_pool.tile([128, KCO, 512], BF16)
                  with tc.tile_pool(name="pt_sb", bufs=6) as pt_pool, \
                     tc.tile_pool(name="att_st", bufs=2) as att_st, \
                     tc.tile_pool(name="st_ps", bufs=4, space="PSUM") as st_ps, \
                     tc.tile_pool(name="l_ps", bufs=2, space="PSUM") as l_ps, \
                     tc.tile_pool(name="o_ps", bufs=2, space="PSUM") as o_ps:
                    for hl in range(HPC):
                        for b in range(B):
                            for qc in range(SB):
                                tcq = b * SB + qc
                                nkb = 4 * qc + 4
                                op = o_ps.tile([128, 512], F32, name="op",
                                               tag="op")
                                lp = l_ps.tile([1, 512], F32, name="lp",
                                               tag="lp")

                                # software-pipelined: lp/op of block kb are
                                # emitted after the score matmuls of kb+1 so
                                # the PE never waits on the scalar-engine exp
                                def lp_op(kb, pt):
                                    nc.tensor.matmul(lp[:], onescb_sb[:],
                                                     pt[:],
                                                     start=(kb == 0),
                                                     stop=(kb == nkb - 1))
                                    nc.tensor.matmul(
                                        op[:],
                                        v_sb[:, b * NKB + kb,
                                             hl * VHD:(hl + 1) * VHD],
                                        pt[:],
                                        start=(kb == 0), stop=(kb == nkb - 1))

                                prev = None
                                for kb in range(nkb):
                                    tck = b * SB + kb // 4
                                    co = (kb % 4) * 128
                                    st = st_ps.tile([128, 512], F32,
                                                    name="st", tag="st")
                                    diag = kb >= 4 * qc
                                    nc.tensor.matmul(
                                        st[:],
                                        knT[:, hl, tck, co:co + 128],
                                        qT[:, hl, tcq, :],
                                        start=True, stop=False)
                                    nc.tensor.matmul(
                                        st[:],
                                        krT[:, tck, co:co + 128],
                                        qTr2[:, hl, tcq, :],
                                        start=False, stop=True)
                                    if prev is not None:
                                        lp_op(*prev)
                                    if diag:
                                        nc.vector.tensor_add(
                                            st[:], st[:],
                                            maskb_sb[:, kb - 4 * qc, :])
                                    pt = pt_pool.tile([128, 512], BF16,
                                                      name="pt", tag="pt")
                                    nc.scalar.activation(pt[:], st[:], AF.Exp,
                                                         scale=SCALE)
                                    prev = (kb, pt)
                                lp_op(*prev)
                                invl = att_st.tile([1, 512], F32, name="invl",
                                                   tag="invl")
                                nc.vector.reciprocal_approx_fast(invl[:], lp[:])
                                bi = att_st.tile([128, 512], F32, name="bi",
                                                 tag="bi")
                                nc.gpsimd.partition_broadcast(bi[:], invl[:])
                                att = att_st.tile([128, 512], BF16, name="att",
                                                  tag="att")
                                nc.vector.tensor_mul(att[:], op[:], bi[:])
                                nc.sync.dma_start(
                                    a2a_in[hl][tcq, :, :], att[:])
                        nc.gpsimd.collective_compute(
                            "AllToAll", mybir.AluOpType.bypass,
                            replica_groups=[list(range(NCORES))],
                            ins=[a2a_in[hl].opt()], outs=[a2a_out[hl].opt()])
                        for i in range(NCORES):
                            nc.sync.dma_start(aosb[:, 2 * i + hl, :],
                                              a2a_out[hl][i])

                  # ============ phase E: o_proj (token-parallel) ===========
                  # Two passes: head-0 contraction chunks (depend only on the
                  # first AllToAll) accumulate + evict partials; the head-1
                  # pass re-seeds PSUM from the partials via identity matmul.
                  with tc.tile_pool(name="yp_sb", bufs=1) as yp_pool, \
                       tc.tile_pool(name="yo_sb", bufs=3) as yo_pool, \
                       tc.tile_pool(name="op_ps", bufs=8, space="PSUM") as op_ps:
                    yp = yp_pool.tile([128, HID // 512, 4, 512], F32R)
                    kc_h0 = [2 * i for i in range(NCORES)]
                    kc_h1 = [2 * i + 1 for i in range(NCORES)]
                    for n in range(HID // 512):
                        pso = [op_ps.tile([128, 512], F32, name=f"pso{mt}",
                                          tag="pso") for mt in range(4)]
                        for ki, kc in enumerate(kc_h0):
                            wot = wo_pool.tile([128, 512], BF16, name="wot",
                                               tag="wot")
                            nc.scalar.dma_start(
                                wot[:], wo[kc, :, n * 512:(n + 1) * 512])
                            for mt in range(4):
                                nc.tensor.matmul(
                                    pso[mt][:],
                                    aosb[:, kc, mt * 128:(mt + 1) * 128],
                                    wot[:],
                                    start=(ki == 0), stop=(ki == NCORES - 1))
                        for mt in range(4):
                            nc.scalar.copy(yp[:, n, mt, :], pso[mt][:])
                    for n in range(HID // 512):
                        pso = [op_ps.tile([128, 512], F32, name=f"ps2{mt}",
                                          tag="pso") for mt in range(4)]
                        for mt in range(4):
                            nc.tensor.matmul(pso[mt][:], identr_sb[:],
                                             yp[:, n, mt, :],
                                             start=True, stop=False)
                        for ki, kc in enumerate(kc_h1):
                            wot = wo_pool.tile([128, 512], BF16, name="wot",
                                               tag="wot")
                            nc.scalar.dma_start(
                                wot[:], wo[kc, :, n * 512:(n + 1) * 512])
                            for mt in range(4):
                                nc.tensor.matmul(
                                    pso[mt][:],
                                    aosb[:, kc, mt * 128:(mt + 1) * 128],
                                    wot[:],
                                    start=False, stop=(ki == NCORES - 1))
                        for mt in range(4):
                            yt = yo_pool.tile([128, 512], F32, name="yt",
                                              tag="yt")
                            nc.scalar.copy(yt[:], pso[mt][:])
                            nc.sync.dma_start(
                                y[mt * 128:(mt + 1) * 128,
                                  n * 512:(n + 1) * 512], yt[:])

    nc.compile()
    return nc


def _rot_lhsT(n):
    """lhsT for the interleaved rotate-half as a matmul: out = R @ x,
    R[2i, 2i+1] = -1, R[2i+1, 2i] = +1; matmul computes lhsT.T @ rhs."""
    R = np.zeros((n, n), dtype=np.float32)
    for i in range(n // 2):
        R[2 * i, 2 * i + 1] = -1.0
        R[2 * i + 1, 2 * i] = 1.0
    return np.ascontiguousarray(R.T)


def _prep_inputs(inputs):
    """Host-side sharding/reordering. Returns in_maps (list of 8 dicts)."""
    import ml_dtypes
    bf16 = ml_dtypes.bfloat16

    hs = np.ascontiguousarray(
        np.asarray(inputs["hidden_states"], dtype=np.float32).reshape(T, IN))
    Wq_a = np.asarray(inputs["Wq_a"], dtype=np.float32)
    q_a_ln = np.asarray(inputs["q_a_ln"], dtype=np.float32)
    Wq_b = np.asarray(inputs["Wq_b"], dtype=np.float32) * q_a_ln[:, None]
    Wkv_a = np.asarray(inputs["Wkv_a"], dtype=np.float32)
    kv_a_ln = np.asarray(inputs["kv_a_ln"], dtype=np.float32)
    Wkv_b = np.asarray(inputs["Wkv_b"], dtype=np.float32) * kv_a_ln[:, None]
    Wo = np.asarray(inputs["Wo"], dtype=np.float32)
    pos = np.asarray(inputs["position_ids"]).astype(np.float64)   # [B, S]

    # rope tables (doubled pairs): cos[2i] = cos[2i+1] = cos(pos * invf_i)
    invf = 1.0 / (THETA ** (np.arange(0, ROPE, 2, dtype=np.float64) / ROPE))
    fr = pos[..., None] * invf                       # [B, S, 32]
    cosd = np.repeat(np.cos(fr), 2, axis=-1).astype(np.float32)  # [B, S, 64]
    sind = np.repeat(np.sin(fr), 2, axis=-1).astype(np.float32)
    cosT = np.ascontiguousarray(cosd.reshape(T, ROPE).T)   # [64, T]
    sinT = np.ascontiguousarray(sind.reshape(T, ROPE).T)
    cos_q = np.concatenate([cosT, cosT], axis=0)           # [128, T]
    sin_q = np.concatenate([sinT, sinT], axis=0)

    # causal additive mask biases for diagonal blocks
    maskb = np.zeros((4, 128, 512), dtype=np.float32)
    kl = np.arange(128)[:, None]
    ql = np.arange(512)[None, :]
    for r in range(4):
        maskb[r] = np.where(kl + 128 * r > ql, NEG, 0.0)

    ident = np.eye(128, dtype=np.float32)
    onesc = np.ones((128, 1), dtype=np.float32)
    onesr = np.ones((1, 128), dtype=np.float32)

    wqa_r = np.ascontiguousarray(
        Wq_a.reshape(IN // 128, 128, QR).astype(bf16))
    wkva_r = np.ascontiguousarray(
        Wkv_a.reshape(IN // 128, 128, KVR + ROPE).astype(bf16))
    wo_r = np.ascontiguousarray(
        Wo.reshape(H * VHD // 128, 128, HID).astype(bf16))

    Wq_b_h = Wq_b.reshape(QR, H, QKD)
    Wkv_b_h = Wkv_b.reshape(KVR, H, NOPE + VHD)

    in_maps = []
    for c in range(NCORES):
        h0, h1 = HPC * c, HPC * c + 1
        bc = c // (NCORES // B)
        s0 = (c % (NCORES // B)) * TSH
        # reorder q_b cols: [nope_h0 | nope_h1 | rope_h0 ; rope_h1]
        wqb_s = np.concatenate([
            Wq_b_h[:, h0, :NOPE], Wq_b_h[:, h1, :NOPE],
            Wq_b_h[:, h0, NOPE:], Wq_b_h[:, h1, NOPE:]], axis=1)
        wqb_s = np.ascontiguousarray(
            wqb_s.reshape(QR // 128, 128, HPC * QKD).astype(bf16))
        wkvb_kn_s = np.ascontiguousarray(
            np.concatenate([Wkv_b_h[:, h0, :NOPE], Wkv_b_h[:, h1, :NOPE]],
                           axis=1).reshape(KVR // 128, 128, HPC * NOPE)
            .astype(bf16))
        wkvb_v_s = np.ascontiguousarray(
            np.concatenate([Wkv_b_h[:, h0, NOPE:], Wkv_b_h[:, h1, NOPE:]],
                           axis=1).reshape(KVR // 128, 128, HPC * VHD)
            .astype(bf16))
        tok0 = c * TSH
        in_maps.append({
            "hid": np.ascontiguousarray(hs[tok0:tok0 + TSH].astype(bf16)),
            "wqa": wqa_r, "wqb": wqb_s, "wkva": wkva_r,
            "wkvb_kn": wkvb_kn_s, "wkvb_v": wkvb_v_s, "wo": wo_r,
            # k-side tables are halved: the rope score matmul contracts the
            # per-head-doubled layout, summing the halved product twice
            "cos_k": np.ascontiguousarray(
                0.5 * cosT[:, bc * S + s0: bc * S + s0 + TSH]),
            "sin_k": np.ascontiguousarray(
                0.5 * sinT[:, bc * S + s0: bc * S + s0 + TSH]),
            "cos_q": cos_q, "sin_q": sin_q,
            "maskb": maskb, "identr": ident, "identb": ident.astype(bf16),
            "r128": np.ascontiguousarray(
                np.block([[_rot_lhsT(ROPE), np.zeros((ROPE, ROPE), np.float32)],
                          [np.zeros((ROPE, ROPE), np.float32), _rot_lhsT(ROPE)]])),
            "r64": _rot_lhsT(ROPE),
            "onesc": onesc, "onescb": onesc.astype(bf16), "onesr": onesr,
        })
    return in_maps


def kernel(**inputs) -> np.ndarray:
    from concourse.bass_utils import run_bass_kernel_spmd

    if "nc" not in _cache:
        _cache["nc"] = _build()
    nc = _cache["nc"]
    in_maps = _prep_inputs(inputs)
    res = run_bass_kernel_spmd(nc, in_maps, core_ids=list(range(NCORES)))
    out = np.concatenate([res.results[c]["y"] for c in range(NCORES)], axis=0)
    return np.ascontiguousarray(out.reshape(B, S, HID))


if __name__ == "__main__":
    rng = np.random.default_rng(0)
    ins = {
        "hidden_states": rng.standard_normal((B, S, IN), dtype=np.float32),
        "Wq_a": rng.standard_normal((IN, QR), dtype=np.float32) * IN ** -0.5,
        "q_a_ln": np.ones(QR, np.float32),
        "Wq_b": rng.standard_normal((QR, H * QKD), dtype=np.float32) * QR ** -0.5,
        "Wkv_a": rng.standard_normal((IN, KVR + ROPE), dtype=np.float32) * IN ** -0.5,
        "kv_a_ln": np.ones(KVR, np.float32),
        "Wkv_b": rng.standard_normal((KVR, H * (NOPE + VHD)), dtype=np.float32) * KVR ** -0.5,
        "Wo": rng.standard_normal((H * VHD, HID), dtype=np.float32) * (H * VHD) ** -0.5,
        "position_ids": np.tile(np.arange(S, dtype=np.int32)[None], (B, 1)),
    }
    out = kernel(**ins)
    print("kernel ran, out shape", out.shape, "absmax", np.abs(out).max())
